# revision 6
# baseline (speedup 1.0000x reference)
"""Trainium2 Bass kernel for Longformer self-attention (B=2, S=4096, D=768, H=12, HD=64, W=256, G=32).

Sharding: 8 cores = 2 batches x 4 head-groups (3 heads each). Each core computes its
batch's projections restricted to its 192 output channels, runs banded + global
attention for its 3 heads.

v1 design (vs v0 baseline at ~248us):
 - hidden_states pre-transposed on host -> contiguous DMA loads (no DMA_TRANSPOSE).
 - q/k/kg projections packed into 5 128-col passes/kt instead of 6 (the three 64-col
   remainders share two passes; t2a holds kg-h2 rows 0:64 + q-h2 rows 64:128, t2b
   holds k-h2 rows 64:128).
 - v/vg bias via broadcast tensor_add on the PSUM->SBUF copy (no ones-matmul).
 - band scores in two f32 PSUM pieces (512+128 cols, 1 bank each); window masking
   done POST-exp as 0/1 multiplies on GpSimd (frees Vector+PE).
 - PV in natural layout: probs chunks [128 keys,128 q] are the stationary operand,
   v_nat [128,65] streams (FD=65); output accumulates per 128-query block in PSUM
   [128, 3*65] including the softmax denominator via the ones column.
 - sg (global-key) and phase-C (global-query) score matmuls pack heads 0+1 into one
   block-diagonal 128-row pass; exp_sg/probs_g are [96, S] (rows 32h..32h+32 = head h).
 - phase C (probs transpose + qg x kg scores + go accumulation) runs incrementally;
   band-score units are interleaved between the long projection fills (Scalar exp is
   ~1us/unit vs 0.32us of PE, so units must stay spread), and small LDW-bound matmuls
   (transposes, go) hide their weight loads under long fills.
Host assembles: out[q] = num/den per head; global-query rows replaced from outg.
"""
import numpy as np
import ml_dtypes

import concourse.bass as bass
import concourse.mybir as mybir
import concourse.tile as tile
from concourse import bacc
from concourse.bass_utils import run_bass_kernel_spmd

B, S, D, H, HD = 2, 4096, 768, 12, 64
W = 256
G = 32
SCALE = 1.0 / np.float32(np.sqrt(HD))
KB = 128
NKB = S // KB     # 32 key blocks
NQB = S // KB     # 32 query blocks
NKT = D // 128    # 6
NNT = S // 512    # 8

BF = mybir.dt.bfloat16
F32 = mybir.dt.float32
AF = mybir.ActivationFunctionType
bf16 = ml_dtypes.bfloat16

_cache = {}


def _span(kb):
    # local valid col range [llo, lhi) within the 640-wide band tile of key block kb
    k0 = KB * kb
    qlo, qhi = max(0, k0 - 2 * KB), min(S, k0 + 3 * KB)
    return qlo, qhi, qlo - (k0 - 2 * KB), qhi - (k0 - 2 * KB)


def _build():
    nc = bacc.Bacc(None, target_bir_lowering=False)

    hsT_d = nc.declare_dram_parameter("hsT", [128, NKT, S], BF, isOutput=False)
    w5_d = nc.declare_dram_parameter("w5", [128, NKT, 5, 128], BF, isOutput=False)
    wv_d = nc.declare_dram_parameter("wv", [128, NKT, 384], BF, isOutput=False)
    wqg_d = nc.declare_dram_parameter("wqg", [128, NKT, 192], BF, isOutput=False)
    biasc_d = nc.declare_dram_parameter("biasc", [128, 8], F32, isOutput=False)
    biasv_d = nc.declare_dram_parameter("biasv", [128, 384], BF, isOutput=False)
    masks_d = nc.declare_dram_parameter("masks01", [128, 256], BF, isOutput=False)
    id96_d = nc.declare_dram_parameter("id96", [96, 96], BF, isOutput=False)
    out_d = nc.declare_dram_parameter("out", [NQB, 128, 3, 65], F32, isOutput=True)
    outg_d = nc.declare_dram_parameter("outg", [96, 65], F32, isOutput=True)

    with tile.TileContext(nc) as tc:
        with tc.tile_pool(name="persist", bufs=1) as pp:
            # --- persistent SBUF ---
            hsT = pp.tile([128, NKT, S], BF)
            qT01 = pp.tile([128, S], BF)
            kT01 = pp.tile([128, S], BF)
            kgT01 = pp.tile([128, S], BF)
            t2a = pp.tile([128, S], BF)   # rows 0:64 kg-h2, rows 64:128 q-h2
            t2b = pp.tile([128, S], BF)   # rows 64:128 k-h2
            v_nat = pp.tile([128, NKB, 3, 65], BF)
            vg_nat = pp.tile([128, NKB, 3, 65], BF)
            exp_sg = pp.tile([96, S], BF)     # rows 32h.. = head h, exp(q . k_glob)
            probs_g = pp.tile([96, S], BF)    # rows 32h.. = head h, exp(qg . kg)
            pb_gT = pp.tile([128, NKB, 96], BF)
            vg3 = pp.tile([96, 65], BF)       # v of global keys, stacked per head
            qgT01 = pp.tile([128, G], BF)
            qgT2b = pp.tile([64, G], BF)
            sgw01 = pp.tile([128, 64], BF)    # block-diag k[:, :G] heads 0|1
            qgw01 = pp.tile([128, 64], BF)    # block-diag qg heads 0|1
            go_acc = pp.tile([96, 65], F32)

            w5_t = pp.tile([128, NKT, 5, 128], BF)
            wv_t = pp.tile([128, NKT, 384], BF)
            wqg_t = pp.tile([128, NKT, 192], BF)
            biasc_t = pp.tile([128, 8], F32)
            biasv_t = pp.tile([128, 384], BF)
            masks_t = pp.tile([128, 256], BF)
            id96_t = pp.tile([96, 96], BF)

            nc.vector.memset(v_nat[:, :, :, 64:65], 1.0)
            nc.vector.memset(vg_nat[:, :, :, 64:65], 1.0)
            nc.vector.memset(vg3[:, 64:65], 1.0)
            nc.vector.memset(sgw01[:], 0.0)
            nc.vector.memset(qgw01[:], 0.0)
            nc.vector.memset(go_acc[:], 0.0)

            # --- input DMAs (sync queue, in-order): small consts, then kt-interleaved
            # weights+first hidden chunk so the first matmul starts ASAP ---
            nc.sync.dma_start(biasc_t[:], biasc_d[:])
            nc.sync.dma_start(masks_t[:], masks_d[:])
            nc.sync.dma_start(id96_t[:], id96_d[:])
            nc.sync.dma_start(biasv_t[:], biasv_d[:])
            for kt in range(NKT):
                nc.sync.dma_start(w5_t[:, kt], w5_d[:, kt])
                nc.sync.dma_start(hsT[:, kt, 0:512], hsT_d[:, kt, 0:512])
            for kt in range(NKT):
                nc.sync.dma_start(wv_t[:, kt], wv_d[:, kt])
            nc.sync.dma_start(wqg_t[:], wqg_d[:])
            for nt in range(1, NNT):
                for kt in range(NKT):
                    c0 = 512 * nt
                    nc.sync.dma_start(hsT[:, kt, c0:c0 + 512], hsT_d[:, kt, c0:c0 + 512])

            with (
                tc.tile_pool(name="apsum", bufs=2, space="PSUM") as apsum,
                tc.tile_pool(name="spsum", bufs=4, space="PSUM") as spsum,
                tc.tile_pool(name="qpsum", bufs=2, space="PSUM") as qpsum,
                tc.tile_pool(name="pbt", bufs=28) as pbtp,
                tc.tile_pool(name="osb", bufs=4) as osbp,
            ):
                pbt = {}

                def qh(h):  # q of head h: (tile, row offset)
                    return (qT01, 64 * h) if h < 2 else (t2a, 64)

                def kh(h):
                    return (kT01, 64 * h) if h < 2 else (t2b, 64)

                def unit(kb, h, smalls):
                    # band scores for one (key block, head): two f32 PSUM pieces,
                    # exp -> bf16 SBUF probs, post-exp 0/1 masking on GpSimd.
                    k0 = KB * kb
                    qlo, qhi, llo, lhi = _span(kb)
                    hiA = min(lhi, 512)
                    kt_, ko = kh(h)
                    qt_, qo = qh(h)
                    psA = spsum.tile([128, 512], F32, tag="sc", name=f"sA{kb}_{h}")
                    nc.tensor.matmul(
                        psA[:, llo:hiA],
                        kt_[ko:ko + 64, k0:k0 + KB],
                        qt_[qo:qo + 64, qlo:qlo + (hiA - llo)])
                    for _ in range(2):
                        if smalls:
                            smalls.pop(0)()
                    t_ = pbtp.tile([128, 640], BF, tag="pb")
                    if lhi > 512:
                        psB = spsum.tile([128, 128], F32, tag="sc", name=f"sB{kb}_{h}")
                        nc.tensor.matmul(
                            psB[:, 0:lhi - 512],
                            kt_[ko:ko + 64, k0:k0 + KB],
                            qt_[qo:qo + 64, qlo + (512 - llo):qlo + (lhi - llo)])
                        nc.scalar.activation(t_[:, 512:lhi], psB[:, 0:lhi - 512], AF.Exp)
                    nc.scalar.activation(t_[:, llo:hiA], psA[:, llo:hiA], AF.Exp)
                    if llo == 0:
                        nc.gpsimd.tensor_mul(t_[:, 0:128], t_[:, 0:128],
                                             masks_t[:, 0:128])
                    if lhi == 640:
                        nc.gpsimd.tensor_mul(t_[:, 512:640], t_[:, 512:640],
                                             masks_t[:, 128:256])
                    pbt[(kb, h)] = t_

                def do_qb(i):
                    # natural-layout PV for query block i (128 queries, 3 heads + denom)
                    qps = qpsum.tile([128, 195], F32, tag="qp", name=f"qp{i}")
                    kbs = list(range(max(0, i - 2), min(NKB - 1, i + 2) + 1))
                    for h in range(3):
                        for n_, j in enumerate(kbs):
                            la = 128 * (i - j) + 256
                            nc.tensor.matmul(
                                qps[:, 65 * h:65 * h + 65],
                                pbt[(j, h)][:, la:la + 128],
                                v_nat[:, j, h, :],
                                start=(n_ == 0), stop=False)
                        nc.tensor.matmul(
                            qps[:, 65 * h:65 * h + 65],
                            exp_sg[32 * h:32 * h + 32, 128 * i:128 * i + 128],
                            vg3[32 * h:32 * h + 32, :],
                            start=False, stop=True)
                    ob = osbp.tile([128, 195], F32, tag="ob")
                    nc.vector.tensor_copy(ob[:], qps[:])
                    nc.gpsimd.dma_start(out_d[i], ob[:].rearrange("p (h e) -> p h e", h=3))

                do_qb.next = 0

                for nt in range(NNT):
                    c0 = 512 * nt
                    # work to interleave into this round:
                    # - early band units: kb in [4nt-4, 4nt-3] (span needs only cols
                    #   < 512nt, i.e. previous rounds' projections)
                    # - late band units: kb in [4nt-2, 4nt-1] (+28..31 at nt=7), after
                    #   this round's q/k/kg passes are copied
                    early = [(kb, h) for kb in range(max(0, 4 * nt - 4), 4 * nt - 2)
                             for h in range(3) if 0 <= kb < NKB]
                    late = [(kb, h)
                            for kb in range(max(0, 4 * nt - 2),
                                            NKB if nt == NNT - 1 else 4 * nt)
                            for h in range(3)]
                    # small LDW-bound matmuls (probs_g transposes of this round's
                    # chunks + go accumulation over last round's chunks)
                    smalls = []

                    def mk_tr(t):
                        def f():
                            pst = apsum.tile([128, 96], BF, tag="pp", name=f"tr{t}")
                            nc.tensor.transpose(pst[:], probs_g[:, 128 * t:128 * t + 128],
                                                id96_t[:])
                            nc.vector.tensor_copy(pb_gT[:, t, :], pst[:])
                        return f

                    gps = None
                    if nt > 0:
                        gps = qpsum.tile([96, 65], F32, tag="qp", name=f"go{nt}")

                        def mk_go(h, t, n_):
                            def f():
                                nc.tensor.matmul(gps[32 * h:32 * h + 32, :],
                                                 pb_gT[:, t, 32 * h:32 * h + 32],
                                                 vg_nat[:, t, h, :],
                                                 start=(n_ == 0), stop=(n_ == 3))
                            return f

                        for h in range(3):
                            for n_, t in enumerate(range(4 * (nt - 1), 4 * nt)):
                                smalls.append(mk_go(h, t, n_))

                    # ---- projections: 5 packed passes, 6 kt each; early band units
                    # and smalls interleaved between the long fills ----
                    dests = [(qT01, 0), (kT01, 1), (kgT01, 2), (t2a, 3), (t2b, 4)]
                    for p, (dst, bc) in enumerate(dests):
                        ps = apsum.tile([128, 512], F32, tag="pp")
                        for kt in range(NKT):
                            nc.tensor.matmul(ps[:], w5_t[:, kt, p, :],
                                             hsT[:, kt, c0:c0 + 512],
                                             start=(kt == 0), stop=(kt == NKT - 1))
                        if p % 2 == 0:
                            nc.vector.tensor_scalar_add(
                                dst[:, c0:c0 + 512], ps[:], biasc_t[:, bc:bc + 1])
                        else:
                            nc.scalar.activation(
                                dst[:, c0:c0 + 512], ps[:], AF.Identity,
                                bias=biasc_t[:, bc:bc + 1], scale=1.0)
                        if smalls:
                            smalls.pop(0)()
                        if early:
                            kb, h = early.pop(0)
                            unit(kb, h, smalls)
                    # ---- v / vg (natural layout), bias via broadcast add ----
                    for s4 in range(4):
                        sb = 4 * nt + s4
                        psv = apsum.tile([128, 384], F32, tag="pp")
                        for kt in range(NKT):
                            nc.tensor.matmul(psv[:],
                                             hsT[:, kt, c0 + 128 * s4:c0 + 128 * s4 + 128],
                                             wv_t[:, kt, :],
                                             start=(kt == 0), stop=(kt == NKT - 1))
                        nc.vector.tensor_add(
                            v_nat[:, sb, :, 0:64],
                            psv[:, 0:192].rearrange("p (h e) -> p h e", h=3),
                            biasv_t[:, 0:192].rearrange("p (h e) -> p h e", h=3))
                        nc.vector.tensor_add(
                            vg_nat[:, sb, :, 0:64],
                            psv[:, 192:384].rearrange("p (h e) -> p h e", h=3),
                            biasv_t[:, 192:384].rearrange("p (h e) -> p h e", h=3))
                        if smalls:
                            smalls.pop(0)()
                        if early:
                            kb, h = early.pop(0)
                            unit(kb, h, smalls)
                        elif late and s4 >= 2:
                            kb, h = late.pop(0)
                            unit(kb, h, smalls)
                    if nt == 0:
                        # qg projection (only first G columns of the sequence)
                        psq = apsum.tile([128, G], F32, tag="pp")
                        for kt in range(NKT):
                            nc.tensor.matmul(psq[:], wqg_t[:, kt, 0:128],
                                             hsT[:, kt, 0:G],
                                             start=(kt == 0), stop=(kt == NKT - 1))
                        nc.vector.tensor_scalar_add(qgT01[:], psq[:], biasc_t[:, 5:6])
                        psq2 = apsum.tile([128, G], F32, tag="pp")
                        for kt in range(NKT):
                            nc.tensor.matmul(psq2[0:64, :], wqg_t[:, kt, 128:192],
                                             hsT[:, kt, 0:G],
                                             start=(kt == 0), stop=(kt == NKT - 1))
                        nc.vector.tensor_scalar_add(qgT2b[:], psq2[0:64, :],
                                                    biasc_t[0:64, 6:7])
                        # block-diagonal packed weights for sg and phase-C scores
                        nc.vector.tensor_copy(sgw01[0:64, 0:32], kT01[0:64, 0:G])
                        nc.vector.tensor_copy(sgw01[64:128, 32:64], kT01[64:128, 0:G])
                        nc.vector.tensor_copy(qgw01[0:64, 0:32], qgT01[0:64, 0:G])
                        nc.vector.tensor_copy(qgw01[64:128, 32:64], qgT01[64:128, 0:G])
                        # v of the G global keys stacked at rows 32h (SBUF->SBUF DMA)
                        for h in range(3):
                            nc.sync.dma_start(vg3[32 * h:32 * h + 32, 0:64],
                                              v_nat[0:G, 0, h, 0:64])
                    # ---- sg: all queries vs G global keys (heads packed 0+1 | 2) ----
                    psS = apsum.tile([96, 512], F32, tag="pp")
                    nc.tensor.matmul(psS[0:64, :], sgw01[:], qT01[:, c0:c0 + 512])
                    nc.tensor.matmul(psS[64:96, :], t2b[64:128, 0:G],
                                     t2a[64:128, c0:c0 + 512])
                    nc.scalar.activation(exp_sg[:, c0:c0 + 512], psS[:], AF.Exp)
                    if late:
                        kb, h = late.pop(0)
                        unit(kb, h, smalls)
                    # ---- phase C scores: G global queries vs this chunk of kg ----
                    psC = apsum.tile([96, 512], F32, tag="pp")
                    nc.tensor.matmul(psC[0:64, :], qgw01[:], kgT01[:, c0:c0 + 512])
                    nc.tensor.matmul(psC[64:96, :], qgT2b[:], t2a[0:64, c0:c0 + 512])
                    nc.scalar.activation(probs_g[:, c0:c0 + 512], psC[:], AF.Exp)
                    for t in range(4 * nt, 4 * nt + 4):
                        smalls.append(mk_tr(t))
                    # ---- remaining late band units + leftover smalls ----
                    while late:
                        kb, h = late.pop(0)
                        unit(kb, h, smalls)
                    for fn in smalls:
                        fn()
                    if gps is not None:
                        nc.vector.tensor_add(go_acc[:], go_acc[:], gps[:])
                    # ---- PV query blocks; emitted only after the go-add above so
                    # the qp slot rotation can never deadlock on gps ----
                    qb_hi = (NQB - 3) if nt == NNT - 1 else (4 * nt - 3)
                    while do_qb.next <= qb_hi:
                        do_qb(do_qb.next)
                        do_qb.next += 1

                # ---- tail: remaining query blocks + last go chunks ----
                smalls = []
                gps = qpsum.tile([96, 65], F32, tag="qp", name="gotail")

                def mk_go2(h, t, n_):
                    def f():
                        nc.tensor.matmul(gps[32 * h:32 * h + 32, :],
                                         pb_gT[:, t, 32 * h:32 * h + 32],
                                         vg_nat[:, t, h, :],
                                         start=(n_ == 0), stop=(n_ == 3))
                    return f

                for h in range(3):
                    for n_, t in enumerate(range(28, 32)):
                        smalls.append(mk_go2(h, t, n_))
                while do_qb.next < NQB:
                    do_qb(do_qb.next)
                    do_qb.next += 1
                    for _ in range(2):
                        if smalls:
                            smalls.pop(0)()
                for fn in smalls:
                    fn()
                nc.vector.tensor_add(go_acc[:], go_acc[:], gps[:])
                nc.gpsimd.dma_start(outg_d[:], go_acc[:])

    nc.compile()
    return nc


def _prep_inputs(inputs):
    hs = np.asarray(inputs["hidden_states"], dtype=np.float32)
    j = np.arange(KB)
    p = np.arange(KB)[:, None]
    m_lo = (j[None, :] >= p).astype(np.float32)
    m_hi = (j[None, :] <= p).astype(np.float32)
    masks01 = np.concatenate([m_lo, m_hi], axis=1).astype(bf16)
    id96 = np.eye(96, dtype=bf16)

    maps = []
    for c in range(8):
        b, hg = c // 4, c % 4
        cols = slice(192 * hg, 192 * hg + 192)
        Wq = np.asarray(inputs["Wq"], np.float32)[:, cols] * SCALE
        bq = np.asarray(inputs["bq"], np.float32)[cols] * SCALE
        Wqg = np.asarray(inputs["Wqg"], np.float32)[:, cols] * SCALE
        bqg = np.asarray(inputs["bqg"], np.float32)[cols] * SCALE
        Wk = np.asarray(inputs["Wk"], np.float32)[:, cols]
        bk = np.asarray(inputs["bk"], np.float32)[cols]
        Wkg = np.asarray(inputs["Wkg"], np.float32)[:, cols]
        bkg = np.asarray(inputs["bkg"], np.float32)[cols]
        Wv = np.asarray(inputs["Wv"], np.float32)[:, cols]
        bv = np.asarray(inputs["bv"], np.float32)[cols]
        Wvg = np.asarray(inputs["Wvg"], np.float32)[:, cols]
        bvg = np.asarray(inputs["bvg"], np.float32)[cols]

        # hidden transposed: [128, kt, s]
        hsT = np.ascontiguousarray(
            hs[b].T.reshape(NKT, 128, S).transpose(1, 0, 2)).astype(bf16)

        # packed q/k/kg weight passes: [128, kt, pass, 128]
        def ktview(Wm):
            return Wm.reshape(NKT, 128, 192)
        Wqk, Wkk, Wgk = ktview(Wq), ktview(Wk), ktview(Wkg)
        w5 = np.zeros((128, NKT, 5, 128), np.float32)
        for kt in range(NKT):
            w5[:, kt, 0, :] = Wqk[kt, :, 0:128]
            w5[:, kt, 1, :] = Wkk[kt, :, 0:128]
            w5[:, kt, 2, :] = Wgk[kt, :, 0:128]
            w5[:, kt, 3, 0:64] = Wgk[kt, :, 128:192]
            w5[:, kt, 3, 64:128] = Wqk[kt, :, 128:192]
            w5[:, kt, 4, 64:128] = Wkk[kt, :, 128:192]
        w5 = w5.astype(bf16)

        wv = np.ascontiguousarray(np.concatenate([Wv, Wvg], axis=1)
                                  .reshape(NKT, 128, 384).transpose(1, 0, 2)).astype(bf16)
        wqg = np.ascontiguousarray(Wqg.reshape(NKT, 128, 192)
                                   .transpose(1, 0, 2)).astype(bf16)

        biasc = np.zeros((128, 8), np.float32)
        biasc[:, 0] = bq[0:128]
        biasc[:, 1] = bk[0:128]
        biasc[:, 2] = bkg[0:128]
        biasc[0:64, 3] = bkg[128:192]
        biasc[64:128, 3] = bq[128:192]
        biasc[64:128, 4] = bk[128:192]
        biasc[:, 5] = bqg[0:128]
        biasc[0:64, 6] = bqg[128:192]

        biasv = np.tile(np.concatenate([bv, bvg])[None, :], (128, 1)).astype(bf16)

        maps.append({
            "hsT": hsT,
            "w5": w5,
            "wv": wv,
            "wqg": wqg,
            "biasc": biasc,
            "biasv": biasv,
            "masks01": masks01,
            "id96": id96,
        })
    return maps


def kernel(**inputs):
    g = int(np.asarray(inputs["num_global"]))
    assert g == G, f"kernel compiled for num_global=32, got {g}"
    if "nc" not in _cache:
        _cache["nc"] = _build()
    nc = _cache["nc"]
    in_maps = _prep_inputs(inputs)
    res = run_bass_kernel_spmd(nc, in_maps, list(range(8)))
    return assemble(res.results)


def assemble(results):
    out = np.zeros((B, S, D), np.float32)
    for c in range(8):
        b, hg = c // 4, c % 4
        o = results[c]["out"].reshape(S, 3, 65)   # natural layout
        og = results[c]["outg"]                   # [96, 65]
        for h in range(3):
            col = 192 * hg + 64 * h
            out[b, :, col:col + 64] = o[:, h, 0:64] / o[:, h, 64:65]
            out[b, 0:G, col:col + 64] = (og[32 * h:32 * h + 32, 0:64]
                                         / og[32 * h:32 * h + 32, 64:65])
    return out


# revision 10
# speedup vs baseline: 1.0213x; 1.0213x over previous
"""Trainium2 Bass kernel for Longformer self-attention (B=2, S=4096, D=768, H=12, HD=64, W=256, G=32).

Sharding: 8 cores = 2 batches x 4 head-groups (3 heads each). Each core computes its
batch's projections restricted to its 192 output channels, runs banded + global
attention for its 3 heads.

v1 design (vs v0 baseline at ~248us):
 - hidden_states pre-transposed on host -> contiguous DMA loads (no DMA_TRANSPOSE).
 - q/k/kg projections packed into 5 128-col passes/kt instead of 6 (the three 64-col
   remainders share two passes; t2a holds kg-h2 rows 0:64 + q-h2 rows 64:128, t2b
   holds k-h2 rows 64:128).
 - v/vg bias via broadcast tensor_add on the PSUM->SBUF copy (no ones-matmul).
 - band scores in two f32 PSUM pieces (512+128 cols, 1 bank each); window masking
   done POST-exp as 0/1 multiplies on GpSimd (frees Vector+PE).
 - PV in natural layout: probs chunks [128 keys,128 q] are the stationary operand,
   v_nat [128,65] streams (FD=65); output accumulates per 128-query block in PSUM
   [128, 3*65] including the softmax denominator via the ones column.
 - sg (global-key) and phase-C (global-query) score matmuls pack heads 0+1 into one
   block-diagonal 128-row pass; exp_sg/probs_g are [96, S] (rows 32h..32h+32 = head h).
 - phase C (probs transpose + qg x kg scores + go accumulation) runs incrementally;
   band-score units are interleaved between the long projection fills (Scalar exp is
   ~1us/unit vs 0.32us of PE, so units must stay spread), and small LDW-bound matmuls
   (transposes, go) hide their weight loads under long fills.
Host assembles: out[q] = num/den per head; global-query rows replaced from outg.
"""
import numpy as np
import ml_dtypes

import concourse.bass as bass
import concourse.mybir as mybir
import concourse.tile as tile
from concourse import bacc
from concourse.bass_utils import run_bass_kernel_spmd

B, S, D, H, HD = 2, 4096, 768, 12, 64
W = 256
G = 32
SCALE = 1.0 / np.float32(np.sqrt(HD))
KB = 128
NKB = S // KB     # 32 key blocks
NQB = S // KB     # 32 query blocks
NKT = D // 128    # 6
NNT = S // 512    # 8

BF = mybir.dt.bfloat16
F32 = mybir.dt.float32
AF = mybir.ActivationFunctionType
bf16 = ml_dtypes.bfloat16

_cache = {}


def _span(kb):
    # local valid col range [llo, lhi) within the 640-wide band tile of key block kb
    k0 = KB * kb
    qlo, qhi = max(0, k0 - 2 * KB), min(S, k0 + 3 * KB)
    return qlo, qhi, qlo - (k0 - 2 * KB), qhi - (k0 - 2 * KB)


def _build():
    nc = bacc.Bacc(None, target_bir_lowering=False)

    hsT_d = nc.declare_dram_parameter("hsT", [128, NKT, S], BF, isOutput=False)
    w5_d = nc.declare_dram_parameter("w5", [128, NKT, 5, 128], BF, isOutput=False)
    wv_d = nc.declare_dram_parameter("wv", [128, NKT, 384], BF, isOutput=False)
    wqg_d = nc.declare_dram_parameter("wqg", [128, NKT, 192], BF, isOutput=False)
    biasc_d = nc.declare_dram_parameter("biasc", [128, 8], F32, isOutput=False)
    biasv_d = nc.declare_dram_parameter("biasv", [128, 384], BF, isOutput=False)
    masks_d = nc.declare_dram_parameter("masks01", [128, 256], BF, isOutput=False)
    id96_d = nc.declare_dram_parameter("id96", [96, 96], BF, isOutput=False)
    out_d = nc.declare_dram_parameter("out", [NQB, 128, 3, 65], F32, isOutput=True)
    outg_d = nc.declare_dram_parameter("outg", [96, 65], F32, isOutput=True)

    with tile.TileContext(nc) as tc:
        with tc.tile_pool(name="persist", bufs=1) as pp:
            # --- persistent SBUF ---
            hsT = pp.tile([128, NKT, S], BF)
            qT01 = pp.tile([128, S], BF)
            kT01 = pp.tile([128, S], BF)
            kgT01 = pp.tile([128, S], BF)
            t2a = pp.tile([128, S], BF)   # rows 0:64 kg-h2, rows 64:128 q-h2
            t2b = pp.tile([128, S], BF)   # rows 64:128 k-h2
            v_nat = pp.tile([128, NKB, 3, 65], BF)
            vg_nat = pp.tile([128, NKB, 3, 65], BF)
            exp_sg = pp.tile([96, S], BF)     # rows 32h.. = head h, exp(q . k_glob)
            probs_g = pp.tile([96, S], BF)    # rows 32h.. = head h, exp(qg . kg)
            pb_gT = pp.tile([128, NKB, 96], BF)
            vg3 = pp.tile([96, 65], BF)       # v of global keys, stacked per head
            qgT01 = pp.tile([128, G], BF)
            qgT2b = pp.tile([64, G], BF)
            sgw01 = pp.tile([128, 64], BF)    # block-diag k[:, :G] heads 0|1
            qgw01 = pp.tile([128, 64], BF)    # block-diag qg heads 0|1
            go_acc = pp.tile([96, 65], F32)

            w5_t = pp.tile([128, NKT, 5, 128], BF)
            wv_t = pp.tile([128, NKT, 384], BF)
            wqg_t = pp.tile([128, NKT, 192], BF)
            biasc_t = pp.tile([128, 8], F32)
            biasv_t = pp.tile([128, 384], BF)
            masks_t = pp.tile([128, 256], BF)
            id96_t = pp.tile([96, 96], BF)

            nc.vector.memset(v_nat[:, :, :, 64:65], 1.0)
            nc.vector.memset(vg_nat[:, :, :, 64:65], 1.0)
            nc.vector.memset(vg3[:, 64:65], 1.0)
            nc.vector.memset(sgw01[:], 0.0)
            nc.vector.memset(qgw01[:], 0.0)
            nc.vector.memset(go_acc[:], 0.0)

            # --- input DMAs on two parallel queues: hidden stream on sync,
            # weights/consts on gpsimd (which later only carries output DMAs) ---
            nc.gpsimd.dma_start(biasc_t[:], biasc_d[:])
            for kt in range(NKT):
                nc.gpsimd.dma_start(w5_t[:, kt], w5_d[:, kt])
            for kt in range(NKT):
                nc.gpsimd.dma_start(wv_t[:, kt], wv_d[:, kt])
            nc.gpsimd.dma_start(biasv_t[:], biasv_d[:])
            nc.gpsimd.dma_start(wqg_t[:], wqg_d[:])
            nc.gpsimd.dma_start(masks_t[:], masks_d[:])
            nc.gpsimd.dma_start(id96_t[:], id96_d[:])
            for nt in range(NNT):
                for kt in range(NKT):
                    c0 = 512 * nt
                    nc.sync.dma_start(hsT[:, kt, c0:c0 + 512], hsT_d[:, kt, c0:c0 + 512])

            with (
                tc.tile_pool(name="apsum", bufs=2, space="PSUM") as apsum,
                tc.tile_pool(name="spsum", bufs=4, space="PSUM") as spsum,
                tc.tile_pool(name="qpsum", bufs=2, space="PSUM") as qpsum,
                tc.tile_pool(name="pbt", bufs=28) as pbtp,
                tc.tile_pool(name="osb", bufs=4) as osbp,
            ):
                pbt = {}

                def qh(h):  # q of head h: (tile, row offset)
                    return (qT01, 64 * h) if h < 2 else (t2a, 64)

                def kh(h):
                    return (kT01, 64 * h) if h < 2 else (t2b, 64)

                def unit(kb, h, smalls):
                    # band scores for one (key block, head): two f32 PSUM pieces,
                    # exp -> bf16 SBUF probs, post-exp 0/1 masking on GpSimd.
                    k0 = KB * kb
                    qlo, qhi, llo, lhi = _span(kb)
                    hiA = min(lhi, 512)
                    kt_, ko = kh(h)
                    qt_, qo = qh(h)
                    psA = spsum.tile([128, 512], F32, tag="sc", name=f"sA{kb}_{h}")
                    nc.tensor.matmul(
                        psA[:, llo:hiA],
                        kt_[ko:ko + 64, k0:k0 + KB],
                        qt_[qo:qo + 64, qlo:qlo + (hiA - llo)])
                    for _ in range(2):
                        if smalls:
                            smalls.pop(0)()
                    t_ = pbtp.tile([128, 640], BF, tag="pb")
                    if lhi > 512:
                        psB = spsum.tile([128, 128], F32, tag="sc", name=f"sB{kb}_{h}")
                        nc.tensor.matmul(
                            psB[:, 0:lhi - 512],
                            kt_[ko:ko + 64, k0:k0 + KB],
                            qt_[qo:qo + 64, qlo + (512 - llo):qlo + (lhi - llo)])
                        nc.scalar.activation(t_[:, 512:lhi], psB[:, 0:lhi - 512], AF.Exp)
                    nc.scalar.activation(t_[:, llo:hiA], psA[:, llo:hiA], AF.Exp)
                    if llo == 0:
                        nc.vector.tensor_mul(t_[:, 0:128], t_[:, 0:128],
                                             masks_t[:, 0:128])
                    if lhi == 640:
                        nc.vector.tensor_mul(t_[:, 512:640], t_[:, 512:640],
                                             masks_t[:, 128:256])
                    pbt[(kb, h)] = t_

                def do_qb(i):
                    # natural-layout PV for query block i (128 queries, 3 heads + denom)
                    qps = qpsum.tile([128, 195], F32, tag="qp", name=f"qp{i}")
                    kbs = list(range(max(0, i - 2), min(NKB - 1, i + 2) + 1))
                    for h in range(3):
                        for n_, j in enumerate(kbs):
                            la = 128 * (i - j) + 256
                            nc.tensor.matmul(
                                qps[:, 65 * h:65 * h + 65],
                                pbt[(j, h)][:, la:la + 128],
                                v_nat[:, j, h, :],
                                start=(n_ == 0), stop=False)
                        nc.tensor.matmul(
                            qps[:, 65 * h:65 * h + 65],
                            exp_sg[32 * h:32 * h + 32, 128 * i:128 * i + 128],
                            vg3[32 * h:32 * h + 32, :],
                            start=False, stop=True)
                    ob = osbp.tile([128, 195], F32, tag="ob")
                    nc.vector.tensor_copy(ob[:], qps[:])
                    nc.gpsimd.dma_start(out_d[i], ob[:].rearrange("p (h e) -> p h e", h=3))

                do_qb.next = 0

                for nt in range(NNT):
                    c0 = 512 * nt
                    # work to interleave into this round:
                    # - early band units: kb in [4nt-4, 4nt-3] (span needs only cols
                    #   < 512nt, i.e. previous rounds' projections)
                    # - late band units: kb in [4nt-2, 4nt-1] (+28..31 at nt=7), after
                    #   this round's q/k/kg passes are copied
                    early = [(kb, h) for kb in range(max(0, 4 * nt - 4), 4 * nt - 2)
                             for h in range(3) if 0 <= kb < NKB]
                    late = [(kb, h)
                            for kb in range(max(0, 4 * nt - 2),
                                            NKB if nt == NNT - 1 else 4 * nt)
                            for h in range(3)]
                    # small LDW-bound matmuls (probs_g transposes of this round's
                    # chunks + go accumulation over last round's chunks)
                    smalls = []

                    def mk_tr(t):
                        def f():
                            pst = apsum.tile([128, 96], BF, tag="pp", name=f"tr{t}")
                            nc.tensor.transpose(pst[:], probs_g[:, 128 * t:128 * t + 128],
                                                id96_t[:])
                            nc.vector.tensor_copy(pb_gT[:, t, :], pst[:])
                        return f

                    gps = None
                    if nt > 0:
                        gps = qpsum.tile([96, 65], F32, tag="qp", name=f"go{nt}")

                        def mk_go(h, t, n_):
                            def f():
                                nc.tensor.matmul(gps[32 * h:32 * h + 32, :],
                                                 pb_gT[:, t, 32 * h:32 * h + 32],
                                                 vg_nat[:, t, h, :],
                                                 start=(n_ == 0), stop=(n_ == 3))
                            return f

                        for h in range(3):
                            for n_, t in enumerate(range(4 * (nt - 1), 4 * nt)):
                                smalls.append(mk_go(h, t, n_))

                    # ---- projections: 5 packed passes, 6 kt each; early band units
                    # and smalls interleaved between the long fills ----
                    dests = [(qT01, 0), (kT01, 1), (kgT01, 2), (t2a, 3), (t2b, 4)]
                    for p, (dst, bc) in enumerate(dests):
                        ps = apsum.tile([128, 512], F32, tag="pp")
                        for kt in range(NKT):
                            nc.tensor.matmul(ps[:], w5_t[:, kt, p, :],
                                             hsT[:, kt, c0:c0 + 512],
                                             start=(kt == 0), stop=(kt == NKT - 1))
                        if p % 2 == 0:
                            nc.vector.tensor_scalar_add(
                                dst[:, c0:c0 + 512], ps[:], biasc_t[:, bc:bc + 1])
                        else:
                            nc.scalar.activation(
                                dst[:, c0:c0 + 512], ps[:], AF.Identity,
                                bias=biasc_t[:, bc:bc + 1], scale=1.0)
                        if smalls:
                            smalls.pop(0)()
                        if early:
                            kb, h = early.pop(0)
                            unit(kb, h, smalls)
                    # ---- v / vg (natural layout), bias via broadcast add ----
                    for s4 in range(4):
                        sb = 4 * nt + s4
                        psv = apsum.tile([128, 384], F32, tag="pp")
                        for kt in range(NKT):
                            nc.tensor.matmul(psv[:],
                                             hsT[:, kt, c0 + 128 * s4:c0 + 128 * s4 + 128],
                                             wv_t[:, kt, :],
                                             start=(kt == 0), stop=(kt == NKT - 1))
                        nc.vector.tensor_add(
                            v_nat[:, sb, :, 0:64],
                            psv[:, 0:192].rearrange("p (h e) -> p h e", h=3),
                            biasv_t[:, 0:192].rearrange("p (h e) -> p h e", h=3))
                        nc.vector.tensor_add(
                            vg_nat[:, sb, :, 0:64],
                            psv[:, 192:384].rearrange("p (h e) -> p h e", h=3),
                            biasv_t[:, 192:384].rearrange("p (h e) -> p h e", h=3))
                        if smalls:
                            smalls.pop(0)()
                        if early:
                            kb, h = early.pop(0)
                            unit(kb, h, smalls)
                        elif late and s4 >= 2:
                            kb, h = late.pop(0)
                            unit(kb, h, smalls)
                    if nt == 0:
                        # qg projection (only first G columns of the sequence)
                        psq = apsum.tile([128, G], F32, tag="pp")
                        for kt in range(NKT):
                            nc.tensor.matmul(psq[:], wqg_t[:, kt, 0:128],
                                             hsT[:, kt, 0:G],
                                             start=(kt == 0), stop=(kt == NKT - 1))
                        nc.vector.tensor_scalar_add(qgT01[:], psq[:], biasc_t[:, 5:6])
                        psq2 = apsum.tile([128, G], F32, tag="pp")
                        for kt in range(NKT):
                            nc.tensor.matmul(psq2[0:64, :], wqg_t[:, kt, 128:192],
                                             hsT[:, kt, 0:G],
                                             start=(kt == 0), stop=(kt == NKT - 1))
                        nc.vector.tensor_scalar_add(qgT2b[:], psq2[0:64, :],
                                                    biasc_t[0:64, 6:7])
                        # block-diagonal packed weights for sg and phase-C scores
                        nc.vector.tensor_copy(sgw01[0:64, 0:32], kT01[0:64, 0:G])
                        nc.vector.tensor_copy(sgw01[64:128, 32:64], kT01[64:128, 0:G])
                        nc.vector.tensor_copy(qgw01[0:64, 0:32], qgT01[0:64, 0:G])
                        nc.vector.tensor_copy(qgw01[64:128, 32:64], qgT01[64:128, 0:G])
                        # v of the G global keys stacked at rows 32h (SBUF->SBUF DMA)
                        for h in range(3):
                            nc.gpsimd.dma_start(vg3[32 * h:32 * h + 32, 0:64],
                                                v_nat[0:G, 0, h, 0:64])
                    # ---- sg: all queries vs G global keys (heads packed 0+1 | 2) ----
                    psS = apsum.tile([96, 512], F32, tag="pp")
                    nc.tensor.matmul(psS[0:64, :], sgw01[:], qT01[:, c0:c0 + 512])
                    nc.tensor.matmul(psS[64:96, :], t2b[64:128, 0:G],
                                     t2a[64:128, c0:c0 + 512])
                    nc.scalar.activation(exp_sg[:, c0:c0 + 512], psS[:], AF.Exp)
                    if late:
                        kb, h = late.pop(0)
                        unit(kb, h, smalls)
                    # ---- phase C scores: G global queries vs this chunk of kg ----
                    psC = apsum.tile([96, 512], F32, tag="pp")
                    nc.tensor.matmul(psC[0:64, :], qgw01[:], kgT01[:, c0:c0 + 512])
                    nc.tensor.matmul(psC[64:96, :], qgT2b[:], t2a[0:64, c0:c0 + 512])
                    nc.scalar.activation(probs_g[:, c0:c0 + 512], psC[:], AF.Exp)
                    for t in range(4 * nt, 4 * nt + 4):
                        smalls.append(mk_tr(t))
                    # ---- remaining late band units + leftover smalls ----
                    while late:
                        kb, h = late.pop(0)
                        unit(kb, h, smalls)
                    for fn in smalls:
                        fn()
                    if gps is not None:
                        nc.vector.tensor_add(go_acc[:], go_acc[:], gps[:])
                    # ---- PV query blocks; emitted only after the go-add above so
                    # the qp slot rotation can never deadlock on gps ----
                    qb_hi = (NQB - 3) if nt == NNT - 1 else (4 * nt - 3)
                    while do_qb.next <= qb_hi:
                        do_qb(do_qb.next)
                        do_qb.next += 1

                # ---- tail: remaining query blocks + last go chunks ----
                smalls = []
                gps = qpsum.tile([96, 65], F32, tag="qp", name="gotail")

                def mk_go2(h, t, n_):
                    def f():
                        nc.tensor.matmul(gps[32 * h:32 * h + 32, :],
                                         pb_gT[:, t, 32 * h:32 * h + 32],
                                         vg_nat[:, t, h, :],
                                         start=(n_ == 0), stop=(n_ == 3))
                    return f

                for h in range(3):
                    for n_, t in enumerate(range(28, 32)):
                        smalls.append(mk_go2(h, t, n_))
                while do_qb.next < NQB:
                    do_qb(do_qb.next)
                    do_qb.next += 1
                    for _ in range(2):
                        if smalls:
                            smalls.pop(0)()
                for fn in smalls:
                    fn()
                nc.vector.tensor_add(go_acc[:], go_acc[:], gps[:])
                nc.gpsimd.dma_start(outg_d[:], go_acc[:])

    nc.compile()
    return nc


def _prep_inputs(inputs):
    hs = np.asarray(inputs["hidden_states"], dtype=np.float32)
    j = np.arange(KB)
    p = np.arange(KB)[:, None]
    m_lo = (j[None, :] >= p).astype(np.float32)
    m_hi = (j[None, :] <= p).astype(np.float32)
    masks01 = np.concatenate([m_lo, m_hi], axis=1).astype(bf16)
    id96 = np.eye(96, dtype=bf16)

    maps = []
    for c in range(8):
        b, hg = c // 4, c % 4
        cols = slice(192 * hg, 192 * hg + 192)
        Wq = np.asarray(inputs["Wq"], np.float32)[:, cols] * SCALE
        bq = np.asarray(inputs["bq"], np.float32)[cols] * SCALE
        Wqg = np.asarray(inputs["Wqg"], np.float32)[:, cols] * SCALE
        bqg = np.asarray(inputs["bqg"], np.float32)[cols] * SCALE
        Wk = np.asarray(inputs["Wk"], np.float32)[:, cols]
        bk = np.asarray(inputs["bk"], np.float32)[cols]
        Wkg = np.asarray(inputs["Wkg"], np.float32)[:, cols]
        bkg = np.asarray(inputs["bkg"], np.float32)[cols]
        Wv = np.asarray(inputs["Wv"], np.float32)[:, cols]
        bv = np.asarray(inputs["bv"], np.float32)[cols]
        Wvg = np.asarray(inputs["Wvg"], np.float32)[:, cols]
        bvg = np.asarray(inputs["bvg"], np.float32)[cols]

        # hidden transposed: [128, kt, s]
        hsT = np.ascontiguousarray(
            hs[b].T.reshape(NKT, 128, S).transpose(1, 0, 2)).astype(bf16)

        # packed q/k/kg weight passes: [128, kt, pass, 128]
        def ktview(Wm):
            return Wm.reshape(NKT, 128, 192)
        Wqk, Wkk, Wgk = ktview(Wq), ktview(Wk), ktview(Wkg)
        w5 = np.zeros((128, NKT, 5, 128), np.float32)
        for kt in range(NKT):
            w5[:, kt, 0, :] = Wqk[kt, :, 0:128]
            w5[:, kt, 1, :] = Wkk[kt, :, 0:128]
            w5[:, kt, 2, :] = Wgk[kt, :, 0:128]
            w5[:, kt, 3, 0:64] = Wgk[kt, :, 128:192]
            w5[:, kt, 3, 64:128] = Wqk[kt, :, 128:192]
            w5[:, kt, 4, 64:128] = Wkk[kt, :, 128:192]
        w5 = w5.astype(bf16)

        wv = np.ascontiguousarray(np.concatenate([Wv, Wvg], axis=1)
                                  .reshape(NKT, 128, 384).transpose(1, 0, 2)).astype(bf16)
        wqg = np.ascontiguousarray(Wqg.reshape(NKT, 128, 192)
                                   .transpose(1, 0, 2)).astype(bf16)

        biasc = np.zeros((128, 8), np.float32)
        biasc[:, 0] = bq[0:128]
        biasc[:, 1] = bk[0:128]
        biasc[:, 2] = bkg[0:128]
        biasc[0:64, 3] = bkg[128:192]
        biasc[64:128, 3] = bq[128:192]
        biasc[64:128, 4] = bk[128:192]
        biasc[:, 5] = bqg[0:128]
        biasc[0:64, 6] = bqg[128:192]

        biasv = np.tile(np.concatenate([bv, bvg])[None, :], (128, 1)).astype(bf16)

        maps.append({
            "hsT": hsT,
            "w5": w5,
            "wv": wv,
            "wqg": wqg,
            "biasc": biasc,
            "biasv": biasv,
            "masks01": masks01,
            "id96": id96,
        })
    return maps


def kernel(**inputs):
    g = int(np.asarray(inputs["num_global"]))
    assert g == G, f"kernel compiled for num_global=32, got {g}"
    if "nc" not in _cache:
        _cache["nc"] = _build()
    nc = _cache["nc"]
    in_maps = _prep_inputs(inputs)
    res = run_bass_kernel_spmd(nc, in_maps, list(range(8)))
    return assemble(res.results)


def assemble(results):
    out = np.zeros((B, S, D), np.float32)
    for c in range(8):
        b, hg = c // 4, c % 4
        o = results[c]["out"].reshape(S, 3, 65)   # natural layout
        og = results[c]["outg"]                   # [96, 65]
        for h in range(3):
            col = 192 * hg + 64 * h
            out[b, :, col:col + 64] = o[:, h, 0:64] / o[:, h, 64:65]
            out[b, 0:G, col:col + 64] = (og[32 * h:32 * h + 32, 0:64]
                                         / og[32 * h:32 * h + 32, 64:65])
    return out


# revision 14
# speedup vs baseline: 1.0330x; 1.0115x over previous
"""Trainium2 Bass kernel for Longformer self-attention (B=2, S=4096, D=768, H=12, HD=64, W=256, G=32).

Sharding: 8 cores = 2 batches x 4 head-groups (3 heads each). Each core computes its
batch's projections restricted to its 192 output channels, runs banded + global
attention for its 3 heads.

v1 design (vs v0 baseline at ~248us):
 - hidden_states pre-transposed on host -> contiguous DMA loads (no DMA_TRANSPOSE).
 - q/k/kg projections packed into 5 128-col passes/kt instead of 6 (the three 64-col
   remainders share two passes; t2a holds kg-h2 rows 0:64 + q-h2 rows 64:128, t2b
   holds k-h2 rows 64:128).
 - v/vg bias via broadcast tensor_add on the PSUM->SBUF copy (no ones-matmul).
 - band scores in two f32 PSUM pieces (512+128 cols, 1 bank each); window masking
   done POST-exp as 0/1 multiplies on GpSimd (frees Vector+PE).
 - PV in natural layout: probs chunks [128 keys,128 q] are the stationary operand,
   v_nat [128,65] streams (FD=65); output accumulates per 128-query block in PSUM
   [128, 3*65] including the softmax denominator via the ones column.
 - sg (global-key) and phase-C (global-query) score matmuls pack heads 0+1 into one
   block-diagonal 128-row pass; exp_sg/probs_g are [96, S] (rows 32h..32h+32 = head h).
 - phase C (probs transpose + qg x kg scores + go accumulation) runs incrementally;
   band-score units are interleaved between the long projection fills (Scalar exp is
   ~1us/unit vs 0.32us of PE, so units must stay spread), and small LDW-bound matmuls
   (transposes, go) hide their weight loads under long fills.
Host assembles: out[q] = num/den per head; global-query rows replaced from outg.
"""
import numpy as np
import ml_dtypes

import concourse.bass as bass
import concourse.mybir as mybir
import concourse.tile as tile
from concourse import bacc
from concourse.bass_utils import run_bass_kernel_spmd

B, S, D, H, HD = 2, 4096, 768, 12, 64
W = 256
G = 32
SCALE = 1.0 / np.float32(np.sqrt(HD))
KB = 128
NKB = S // KB     # 32 key blocks
NQB = S // KB     # 32 query blocks
NKT = D // 128    # 6
NNT = S // 512    # 8

BF = mybir.dt.bfloat16
F32 = mybir.dt.float32
AF = mybir.ActivationFunctionType
bf16 = ml_dtypes.bfloat16

_cache = {}


def _span(kb):
    # local valid col range [llo, lhi) within the 640-wide band tile of key block kb
    k0 = KB * kb
    qlo, qhi = max(0, k0 - 2 * KB), min(S, k0 + 3 * KB)
    return qlo, qhi, qlo - (k0 - 2 * KB), qhi - (k0 - 2 * KB)


def _build():
    nc = bacc.Bacc(None, target_bir_lowering=False)

    hsT_d = nc.declare_dram_parameter("hsT", [128, NKT, S], BF, isOutput=False)
    w5_d = nc.declare_dram_parameter("w5", [128, NKT, 5, 128], BF, isOutput=False)
    wv_d = nc.declare_dram_parameter("wv", [128, NKT, 384], BF, isOutput=False)
    wqg_d = nc.declare_dram_parameter("wqg", [128, NKT, 192], BF, isOutput=False)
    biasc_d = nc.declare_dram_parameter("biasc", [128, 8], F32, isOutput=False)
    biasv_d = nc.declare_dram_parameter("biasv", [128, 384], BF, isOutput=False)
    masks_d = nc.declare_dram_parameter("masks01", [128, 256], BF, isOutput=False)
    id96_d = nc.declare_dram_parameter("id96", [96, 96], BF, isOutput=False)
    out_d = nc.declare_dram_parameter("out", [NQB, 128, 3, 65], F32, isOutput=True)
    outg_d = nc.declare_dram_parameter("outg", [96, 65], F32, isOutput=True)

    with tile.TileContext(nc) as tc:
        with tc.tile_pool(name="persist", bufs=1) as pp:
            # --- persistent SBUF ---
            hsT = pp.tile([128, NKT, S], BF)
            qT01 = pp.tile([128, S], BF)
            kT01 = pp.tile([128, S], BF)
            kgT01 = pp.tile([128, S], BF)
            t2a = pp.tile([128, S], BF)   # rows 0:64 kg-h2, rows 64:128 q-h2
            t2b = pp.tile([128, S], BF)   # rows 64:128 k-h2
            v_nat = pp.tile([128, NKB, 3, 65], BF)
            vg_nat = pp.tile([128, NKB, 3, 65], BF)
            exp_sg = pp.tile([96, S], BF)     # rows 32h.. = head h, exp(q . k_glob)
            probs_g = pp.tile([96, S], BF)    # rows 32h.. = head h, exp(qg . kg)
            pb_gT = pp.tile([128, NKB, 96], BF)
            vg3 = pp.tile([96, 65], BF)       # v of global keys, stacked per head
            qgT01 = pp.tile([128, G], BF)
            qgT2b = pp.tile([64, G], BF)
            sgw01 = pp.tile([128, 64], BF)    # block-diag k[:, :G] heads 0|1
            qgw01 = pp.tile([128, 64], BF)    # block-diag qg heads 0|1
            go_acc = pp.tile([96, 65], F32)

            w5_t = pp.tile([128, NKT, 5, 128], BF)
            wv_t = pp.tile([128, NKT, 384], BF)
            wqg_t = pp.tile([128, NKT, 192], BF)
            biasc_t = pp.tile([128, 8], F32)
            biasv_t = pp.tile([128, 384], BF)
            masks_t = pp.tile([128, 256], BF)
            id96_t = pp.tile([96, 96], BF)

            nc.vector.memset(v_nat[:, :, :, 64:65], 1.0)
            nc.vector.memset(vg_nat[:, :, :, 64:65], 1.0)
            nc.vector.memset(vg3[:, 64:65], 1.0)
            nc.vector.memset(sgw01[:], 0.0)
            nc.vector.memset(qgw01[:], 0.0)
            nc.vector.memset(go_acc[:], 0.0)

            # --- input DMAs on two parallel queues: hidden stream on sync,
            # weights/consts on gpsimd (which later only carries output DMAs) ---
            nc.gpsimd.dma_start(biasc_t[:], biasc_d[:])
            for kt in range(NKT):
                nc.gpsimd.dma_start(w5_t[:, kt], w5_d[:, kt])
            for kt in range(NKT):
                nc.gpsimd.dma_start(wv_t[:, kt], wv_d[:, kt])
            nc.gpsimd.dma_start(biasv_t[:], biasv_d[:])
            nc.gpsimd.dma_start(wqg_t[:], wqg_d[:])
            nc.gpsimd.dma_start(masks_t[:], masks_d[:])
            nc.gpsimd.dma_start(id96_t[:], id96_d[:])
            for nt in range(NNT):
                for kt in range(NKT):
                    c0 = 512 * nt
                    nc.sync.dma_start(hsT[:, kt, c0:c0 + 512], hsT_d[:, kt, c0:c0 + 512])

            with (
                tc.tile_pool(name="apsum", bufs=2, space="PSUM") as apsum,
                tc.tile_pool(name="spsum", bufs=4, space="PSUM") as spsum,
                tc.tile_pool(name="qpsum", bufs=2, space="PSUM") as qpsum,
                tc.tile_pool(name="pbt", bufs=28) as pbtp,
                tc.tile_pool(name="osb", bufs=4) as osbp,
            ):
                pbt = {}

                def qh(h):  # q of head h: (tile, row offset)
                    return (qT01, 64 * h) if h < 2 else (t2a, 64)

                def kh(h):
                    return (kT01, 64 * h) if h < 2 else (t2b, 64)

                def unit(kb, h, smalls):
                    # band scores for one (key block, head): two f32 PSUM pieces,
                    # exp -> bf16 SBUF probs, post-exp 0/1 masking on GpSimd.
                    k0 = KB * kb
                    qlo, qhi, llo, lhi = _span(kb)
                    hiA = min(lhi, 512)
                    kt_, ko = kh(h)
                    qt_, qo = qh(h)
                    psA = spsum.tile([128, 512], F32, tag="sc", name=f"sA{kb}_{h}")
                    nc.tensor.matmul(
                        psA[:, llo:hiA],
                        kt_[ko:ko + 64, k0:k0 + KB],
                        qt_[qo:qo + 64, qlo:qlo + (hiA - llo)])
                    for _ in range(2):
                        if smalls:
                            smalls.pop(0)()
                    t_ = pbtp.tile([128, 640], BF, tag="pb")
                    if lhi > 512:
                        psB = spsum.tile([128, 128], F32, tag="sc", name=f"sB{kb}_{h}")
                        nc.tensor.matmul(
                            psB[:, 0:lhi - 512],
                            kt_[ko:ko + 64, k0:k0 + KB],
                            qt_[qo:qo + 64, qlo + (512 - llo):qlo + (lhi - llo)])
                        nc.scalar.activation(t_[:, 512:lhi], psB[:, 0:lhi - 512], AF.Exp)
                    nc.scalar.activation(t_[:, llo:hiA], psA[:, llo:hiA], AF.Exp)
                    if llo == 0:
                        nc.gpsimd.tensor_mul(t_[:, 0:128], t_[:, 0:128],
                                             masks_t[:, 0:128])
                    if lhi == 640:
                        nc.gpsimd.tensor_mul(t_[:, 512:640], t_[:, 512:640],
                                             masks_t[:, 128:256])
                    pbt[(kb, h)] = t_

                def do_qb(i):
                    # natural-layout PV for query block i (128 queries, 3 heads + denom)
                    qps = qpsum.tile([128, 195], F32, tag="qp", name=f"qp{i}")
                    kbs = list(range(max(0, i - 2), min(NKB - 1, i + 2) + 1))
                    for h in range(3):
                        for n_, j in enumerate(kbs):
                            la = 128 * (i - j) + 256
                            nc.tensor.matmul(
                                qps[:, 65 * h:65 * h + 65],
                                pbt[(j, h)][:, la:la + 128],
                                v_nat[:, j, h, :],
                                start=(n_ == 0), stop=False)
                        nc.tensor.matmul(
                            qps[:, 65 * h:65 * h + 65],
                            exp_sg[32 * h:32 * h + 32, 128 * i:128 * i + 128],
                            vg3[32 * h:32 * h + 32, :],
                            start=False, stop=True)
                    ob = osbp.tile([128, 195], F32, tag="ob")
                    nc.vector.tensor_copy(ob[:], qps[:])
                    nc.sync.dma_start(out_d[i], ob[:].rearrange("p (h e) -> p h e", h=3))

                do_qb.next = 0

                for nt in range(NNT):
                    c0 = 512 * nt
                    # work to interleave into this round:
                    # - early band units: kb in [4nt-4, 4nt-3] (span needs only cols
                    #   < 512nt, i.e. previous rounds' projections)
                    # - late band units: kb in [4nt-2, 4nt-1] (+28..31 at nt=7), after
                    #   this round's q/k/kg passes are copied
                    early = [(kb, h) for kb in range(max(0, 4 * nt - 4), 4 * nt - 2)
                             for h in range(3) if 0 <= kb < NKB]
                    late = [(kb, h)
                            for kb in range(max(0, 4 * nt - 2),
                                            NKB if nt == NNT - 1 else 4 * nt)
                            for h in range(3)]
                    # small LDW-bound matmuls (probs_g transposes of this round's
                    # chunks + go accumulation over last round's chunks)
                    smalls = []

                    def mk_tr(t):
                        def f():
                            pst = apsum.tile([128, 96], BF, tag="pp", name=f"tr{t}")
                            nc.tensor.transpose(pst[:], probs_g[:, 128 * t:128 * t + 128],
                                                id96_t[:])
                            nc.vector.tensor_copy(pb_gT[:, t, :], pst[:])
                        return f

                    gps = None
                    if nt > 0:
                        gps = qpsum.tile([96, 65], F32, tag="qp", name=f"go{nt}")

                        def mk_go(h, t, n_):
                            def f():
                                nc.tensor.matmul(gps[32 * h:32 * h + 32, :],
                                                 pb_gT[:, t, 32 * h:32 * h + 32],
                                                 vg_nat[:, t, h, :],
                                                 start=(n_ == 0), stop=(n_ == 3))
                            return f

                        for h in range(3):
                            for n_, t in enumerate(range(4 * (nt - 1), 4 * nt)):
                                smalls.append(mk_go(h, t, n_))

                    # ---- projections: 5 packed passes, 6 kt each; early band units
                    # and smalls interleaved between the long fills ----
                    dests = [(qT01, 0), (kT01, 1), (kgT01, 2), (t2a, 3), (t2b, 4)]
                    for p, (dst, bc) in enumerate(dests):
                        ps = apsum.tile([128, 512], F32, tag="pp")
                        for kt in range(NKT):
                            nc.tensor.matmul(ps[:], w5_t[:, kt, p, :],
                                             hsT[:, kt, c0:c0 + 512],
                                             start=(kt == 0), stop=(kt == NKT - 1))
                        nc.vector.tensor_scalar_add(
                            dst[:, c0:c0 + 512], ps[:], biasc_t[:, bc:bc + 1])
                        if smalls:
                            smalls.pop(0)()
                        if early:
                            kb, h = early.pop(0)
                            unit(kb, h, smalls)
                    # ---- v / vg (natural layout), bias via broadcast add ----
                    for s4 in range(4):
                        sb = 4 * nt + s4
                        psv = apsum.tile([128, 384], F32, tag="pp")
                        for kt in range(NKT):
                            nc.tensor.matmul(psv[:],
                                             hsT[:, kt, c0 + 128 * s4:c0 + 128 * s4 + 128],
                                             wv_t[:, kt, :],
                                             start=(kt == 0), stop=(kt == NKT - 1))
                        nc.vector.tensor_add(
                            v_nat[:, sb, :, 0:64],
                            psv[:, 0:192].rearrange("p (h e) -> p h e", h=3),
                            biasv_t[:, 0:192].rearrange("p (h e) -> p h e", h=3))
                        nc.vector.tensor_add(
                            vg_nat[:, sb, :, 0:64],
                            psv[:, 192:384].rearrange("p (h e) -> p h e", h=3),
                            biasv_t[:, 192:384].rearrange("p (h e) -> p h e", h=3))
                        if smalls:
                            smalls.pop(0)()
                        if early:
                            kb, h = early.pop(0)
                            unit(kb, h, smalls)
                        elif late and s4 >= 2:
                            kb, h = late.pop(0)
                            unit(kb, h, smalls)
                    if nt == 0:
                        # qg projection (only first G columns of the sequence)
                        psq = apsum.tile([128, G], F32, tag="pp")
                        for kt in range(NKT):
                            nc.tensor.matmul(psq[:], wqg_t[:, kt, 0:128],
                                             hsT[:, kt, 0:G],
                                             start=(kt == 0), stop=(kt == NKT - 1))
                        nc.vector.tensor_scalar_add(qgT01[:], psq[:], biasc_t[:, 5:6])
                        psq2 = apsum.tile([128, G], F32, tag="pp")
                        for kt in range(NKT):
                            nc.tensor.matmul(psq2[0:64, :], wqg_t[:, kt, 128:192],
                                             hsT[:, kt, 0:G],
                                             start=(kt == 0), stop=(kt == NKT - 1))
                        nc.vector.tensor_scalar_add(qgT2b[:], psq2[0:64, :],
                                                    biasc_t[0:64, 6:7])
                        # block-diagonal packed weights for sg and phase-C scores
                        nc.vector.tensor_copy(sgw01[0:64, 0:32], kT01[0:64, 0:G])
                        nc.vector.tensor_copy(sgw01[64:128, 32:64], kT01[64:128, 0:G])
                        nc.vector.tensor_copy(qgw01[0:64, 0:32], qgT01[0:64, 0:G])
                        nc.vector.tensor_copy(qgw01[64:128, 32:64], qgT01[64:128, 0:G])
                        # v of the G global keys stacked at rows 32h (SBUF->SBUF DMA)
                        for h in range(3):
                            nc.gpsimd.dma_start(vg3[32 * h:32 * h + 32, 0:64],
                                                v_nat[0:G, 0, h, 0:64])
                    # ---- sg: all queries vs G global keys (heads packed 0+1 | 2) ----
                    psS = apsum.tile([96, 512], F32, tag="pp")
                    nc.tensor.matmul(psS[0:64, :], sgw01[:], qT01[:, c0:c0 + 512])
                    nc.tensor.matmul(psS[64:96, :], t2b[64:128, 0:G],
                                     t2a[64:128, c0:c0 + 512])
                    nc.scalar.activation(exp_sg[:, c0:c0 + 512], psS[:], AF.Exp)
                    if late:
                        kb, h = late.pop(0)
                        unit(kb, h, smalls)
                    # ---- phase C scores: G global queries vs this chunk of kg ----
                    psC = apsum.tile([96, 512], F32, tag="pp")
                    nc.tensor.matmul(psC[0:64, :], qgw01[:], kgT01[:, c0:c0 + 512])
                    nc.tensor.matmul(psC[64:96, :], qgT2b[:], t2a[0:64, c0:c0 + 512])
                    nc.scalar.activation(probs_g[:, c0:c0 + 512], psC[:], AF.Exp)
                    for t in range(4 * nt, 4 * nt + 4):
                        smalls.append(mk_tr(t))
                    # ---- remaining late band units + leftover smalls ----
                    while late:
                        kb, h = late.pop(0)
                        unit(kb, h, smalls)
                    for fn in smalls:
                        fn()
                    if gps is not None:
                        nc.vector.tensor_add(go_acc[:], go_acc[:], gps[:])
                    # ---- PV query blocks; emitted only after the go-add above so
                    # the qp slot rotation can never deadlock on gps ----
                    qb_hi = (NQB - 3) if nt == NNT - 1 else (4 * nt - 3)
                    while do_qb.next <= qb_hi:
                        do_qb(do_qb.next)
                        do_qb.next += 1

                # ---- tail: remaining query blocks + last go chunks ----
                smalls = []
                gps = qpsum.tile([96, 65], F32, tag="qp", name="gotail")

                def mk_go2(h, t, n_):
                    def f():
                        nc.tensor.matmul(gps[32 * h:32 * h + 32, :],
                                         pb_gT[:, t, 32 * h:32 * h + 32],
                                         vg_nat[:, t, h, :],
                                         start=(n_ == 0), stop=(n_ == 3))
                    return f

                for h in range(3):
                    for n_, t in enumerate(range(28, 32)):
                        smalls.append(mk_go2(h, t, n_))
                while do_qb.next < NQB:
                    do_qb(do_qb.next)
                    do_qb.next += 1
                    for _ in range(2):
                        if smalls:
                            smalls.pop(0)()
                for fn in smalls:
                    fn()
                nc.vector.tensor_add(go_acc[:], go_acc[:], gps[:])
                nc.sync.dma_start(outg_d[:], go_acc[:])

    nc.compile()
    return nc


def _prep_inputs(inputs):
    hs = np.asarray(inputs["hidden_states"], dtype=np.float32)
    j = np.arange(KB)
    p = np.arange(KB)[:, None]
    m_lo = (j[None, :] >= p).astype(np.float32)
    m_hi = (j[None, :] <= p).astype(np.float32)
    masks01 = np.concatenate([m_lo, m_hi], axis=1).astype(bf16)
    id96 = np.eye(96, dtype=bf16)

    maps = []
    for c in range(8):
        b, hg = c // 4, c % 4
        cols = slice(192 * hg, 192 * hg + 192)
        Wq = np.asarray(inputs["Wq"], np.float32)[:, cols] * SCALE
        bq = np.asarray(inputs["bq"], np.float32)[cols] * SCALE
        Wqg = np.asarray(inputs["Wqg"], np.float32)[:, cols] * SCALE
        bqg = np.asarray(inputs["bqg"], np.float32)[cols] * SCALE
        Wk = np.asarray(inputs["Wk"], np.float32)[:, cols]
        bk = np.asarray(inputs["bk"], np.float32)[cols]
        Wkg = np.asarray(inputs["Wkg"], np.float32)[:, cols]
        bkg = np.asarray(inputs["bkg"], np.float32)[cols]
        Wv = np.asarray(inputs["Wv"], np.float32)[:, cols]
        bv = np.asarray(inputs["bv"], np.float32)[cols]
        Wvg = np.asarray(inputs["Wvg"], np.float32)[:, cols]
        bvg = np.asarray(inputs["bvg"], np.float32)[cols]

        # hidden transposed: [128, kt, s]
        hsT = np.ascontiguousarray(
            hs[b].T.reshape(NKT, 128, S).transpose(1, 0, 2)).astype(bf16)

        # packed q/k/kg weight passes: [128, kt, pass, 128]
        def ktview(Wm):
            return Wm.reshape(NKT, 128, 192)
        Wqk, Wkk, Wgk = ktview(Wq), ktview(Wk), ktview(Wkg)
        w5 = np.zeros((128, NKT, 5, 128), np.float32)
        for kt in range(NKT):
            w5[:, kt, 0, :] = Wqk[kt, :, 0:128]
            w5[:, kt, 1, :] = Wkk[kt, :, 0:128]
            w5[:, kt, 2, :] = Wgk[kt, :, 0:128]
            w5[:, kt, 3, 0:64] = Wgk[kt, :, 128:192]
            w5[:, kt, 3, 64:128] = Wqk[kt, :, 128:192]
            w5[:, kt, 4, 64:128] = Wkk[kt, :, 128:192]
        w5 = w5.astype(bf16)

        wv = np.ascontiguousarray(np.concatenate([Wv, Wvg], axis=1)
                                  .reshape(NKT, 128, 384).transpose(1, 0, 2)).astype(bf16)
        wqg = np.ascontiguousarray(Wqg.reshape(NKT, 128, 192)
                                   .transpose(1, 0, 2)).astype(bf16)

        biasc = np.zeros((128, 8), np.float32)
        biasc[:, 0] = bq[0:128]
        biasc[:, 1] = bk[0:128]
        biasc[:, 2] = bkg[0:128]
        biasc[0:64, 3] = bkg[128:192]
        biasc[64:128, 3] = bq[128:192]
        biasc[64:128, 4] = bk[128:192]
        biasc[:, 5] = bqg[0:128]
        biasc[0:64, 6] = bqg[128:192]

        biasv = np.tile(np.concatenate([bv, bvg])[None, :], (128, 1)).astype(bf16)

        maps.append({
            "hsT": hsT,
            "w5": w5,
            "wv": wv,
            "wqg": wqg,
            "biasc": biasc,
            "biasv": biasv,
            "masks01": masks01,
            "id96": id96,
        })
    return maps


def kernel(**inputs):
    g = int(np.asarray(inputs["num_global"]))
    assert g == G, f"kernel compiled for num_global=32, got {g}"
    if "nc" not in _cache:
        _cache["nc"] = _build()
    nc = _cache["nc"]
    in_maps = _prep_inputs(inputs)
    res = run_bass_kernel_spmd(nc, in_maps, list(range(8)))
    return assemble(res.results)


def assemble(results):
    out = np.zeros((B, S, D), np.float32)
    for c in range(8):
        b, hg = c // 4, c % 4
        o = results[c]["out"].reshape(S, 3, 65)   # natural layout
        og = results[c]["outg"]                   # [96, 65]
        for h in range(3):
            col = 192 * hg + 64 * h
            out[b, :, col:col + 64] = o[:, h, 0:64] / o[:, h, 64:65]
            out[b, 0:G, col:col + 64] = (og[32 * h:32 * h + 32, 0:64]
                                         / og[32 * h:32 * h + 32, 64:65])
    return out


# revision 16
# speedup vs baseline: 1.0337x; 1.0007x over previous
"""Trainium2 Bass kernel for Longformer self-attention (B=2, S=4096, D=768, H=12, HD=64, W=256, G=32).

Sharding: 8 cores = 2 batches x 4 head-groups (3 heads each). Each core computes its
batch's projections restricted to its 192 output channels, runs banded + global
attention for its 3 heads.

v1 design (vs v0 baseline at ~248us):
 - hidden_states pre-transposed on host -> contiguous DMA loads (no DMA_TRANSPOSE).
 - q/k/kg projections packed into 5 128-col passes/kt instead of 6 (the three 64-col
   remainders share two passes; t2a holds kg-h2 rows 0:64 + q-h2 rows 64:128, t2b
   holds k-h2 rows 64:128).
 - v/vg bias via broadcast tensor_add on the PSUM->SBUF copy (no ones-matmul).
 - band scores in two f32 PSUM pieces (512+128 cols, 1 bank each); window masking
   done POST-exp as 0/1 multiplies on GpSimd (frees Vector+PE).
 - PV in natural layout: probs chunks [128 keys,128 q] are the stationary operand,
   v_nat [128,65] streams (FD=65); output accumulates per 128-query block in PSUM
   [128, 3*65] including the softmax denominator via the ones column.
 - sg (global-key) and phase-C (global-query) score matmuls pack heads 0+1 into one
   block-diagonal 128-row pass; exp_sg/probs_g are [96, S] (rows 32h..32h+32 = head h).
 - phase C (probs transpose + qg x kg scores + go accumulation) runs incrementally;
   band-score units are interleaved between the long projection fills (Scalar exp is
   ~1us/unit vs 0.32us of PE, so units must stay spread), and small LDW-bound matmuls
   (transposes, go) hide their weight loads under long fills.
Host assembles: out[q] = num/den per head; global-query rows replaced from outg.
"""
import numpy as np
import ml_dtypes

import concourse.bass as bass
import concourse.mybir as mybir
import concourse.tile as tile
from concourse import bacc
from concourse.bass_utils import run_bass_kernel_spmd

B, S, D, H, HD = 2, 4096, 768, 12, 64
W = 256
G = 32
SCALE = 1.0 / np.float32(np.sqrt(HD))
KB = 128
NKB = S // KB     # 32 key blocks
NQB = S // KB     # 32 query blocks
NKT = D // 128    # 6
NNT = S // 512    # 8

BF = mybir.dt.bfloat16
F32 = mybir.dt.float32
AF = mybir.ActivationFunctionType
bf16 = ml_dtypes.bfloat16

_cache = {}


def _span(kb):
    # local valid col range [llo, lhi) within the 640-wide band tile of key block kb
    k0 = KB * kb
    qlo, qhi = max(0, k0 - 2 * KB), min(S, k0 + 3 * KB)
    return qlo, qhi, qlo - (k0 - 2 * KB), qhi - (k0 - 2 * KB)


def _build():
    nc = bacc.Bacc(None, target_bir_lowering=False)

    hsT_d = nc.declare_dram_parameter("hsT", [128, NKT, S], BF, isOutput=False)
    w5_d = nc.declare_dram_parameter("w5", [128, NKT, 5, 128], BF, isOutput=False)
    wv_d = nc.declare_dram_parameter("wv", [128, NKT, 384], BF, isOutput=False)
    wqg_d = nc.declare_dram_parameter("wqg", [128, NKT, 192], BF, isOutput=False)
    biasc_d = nc.declare_dram_parameter("biasc", [128, 8], F32, isOutput=False)
    biasv_d = nc.declare_dram_parameter("biasv", [128, 384], BF, isOutput=False)
    masks_d = nc.declare_dram_parameter("masksNEG", [128, 256], F32, isOutput=False)
    id128_d = nc.declare_dram_parameter("id128", [128, 128], BF, isOutput=False)
    out_d = nc.declare_dram_parameter("out", [NQB, 128, 3, 65], F32, isOutput=True)
    outg_d = nc.declare_dram_parameter("outg", [96, 65], F32, isOutput=True)

    with tile.TileContext(nc) as tc:
        with tc.tile_pool(name="persist", bufs=1) as pp:
            # --- persistent SBUF ---
            hsT = pp.tile([128, NKT, S], BF)
            qT01 = pp.tile([128, S], BF)
            kT01 = pp.tile([128, S], BF)
            kgT01 = pp.tile([128, S], BF)
            t2a = pp.tile([128, S], BF)   # rows 0:64 kg-h2, rows 64:128 q-h2
            t2b = pp.tile([128, S], BF)   # rows 64:128 k-h2
            v_nat = pp.tile([128, NKB, 3, 65], BF)
            vg_nat = pp.tile([128, NKB, 3, 65], BF)
            exp_sg = pp.tile([96, S], BF)     # rows 32h.. = head h, exp(q . k_glob)
            probs_g = pp.tile([128, S], BF)   # rows 32h.. = head h, exp(qg . kg); rows 96+ zero
            pb_gT = pp.tile([128, NKB, 96], BF)
            vg3 = pp.tile([96, 65], BF)       # v of global keys, stacked per head
            qgT01 = pp.tile([128, G], BF)
            qgT2b = pp.tile([64, G], BF)
            sgw01 = pp.tile([128, 128], BF)   # block-diag k[:, :G] heads 0|1 (padded)
            qgw01 = pp.tile([128, 128], BF)   # block-diag qg heads 0|1 (padded)
            go_acc = pp.tile([96, 65], F32)

            w5_t = pp.tile([128, NKT, 5, 128], BF)
            wv_t = pp.tile([128, NKT, 384], BF)
            wqg_t = pp.tile([128, NKT, 192], BF)
            biasc_t = pp.tile([128, 8], F32)
            biasv_t = pp.tile([128, 384], BF)
            masks_t = pp.tile([128, 256], F32)
            id128_t = pp.tile([128, 128], BF)

            nc.vector.memset(v_nat[:, :, :, 64:65], 1.0)
            nc.vector.memset(vg_nat[:, :, :, 64:65], 1.0)
            nc.vector.memset(vg3[:, 64:65], 1.0)
            nc.vector.memset(sgw01[:], 0.0)
            nc.vector.memset(qgw01[:], 0.0)
            nc.vector.memset(go_acc[:], 0.0)
            nc.vector.memset(probs_g[96:128, :], 0.0)

            # --- input DMAs on two parallel queues: hidden stream on sync,
            # weights/consts on gpsimd (which later only carries output DMAs) ---
            nc.gpsimd.dma_start(biasc_t[:], biasc_d[:])
            for kt in range(NKT):
                nc.gpsimd.dma_start(w5_t[:, kt], w5_d[:, kt])
            for kt in range(NKT):
                nc.gpsimd.dma_start(wv_t[:, kt], wv_d[:, kt])
            nc.gpsimd.dma_start(biasv_t[:], biasv_d[:])
            nc.gpsimd.dma_start(wqg_t[:], wqg_d[:])
            nc.gpsimd.dma_start(masks_t[:], masks_d[:])
            nc.gpsimd.dma_start(id128_t[:], id128_d[:])
            for nt in range(NNT):
                for kt in range(NKT):
                    c0 = 512 * nt
                    nc.sync.dma_start(hsT[:, kt, c0:c0 + 512], hsT_d[:, kt, c0:c0 + 512])

            with (
                tc.tile_pool(name="apsum", bufs=2, space="PSUM") as apsum,
                tc.tile_pool(name="spsum", bufs=4, space="PSUM") as spsum,
                tc.tile_pool(name="qpsum", bufs=2, space="PSUM") as qpsum,
                tc.tile_pool(name="pbt", bufs=28) as pbtp,
                tc.tile_pool(name="osb", bufs=4) as osbp,
            ):
                pbt = {}

                def qh(h):  # q of head h: (tile, row offset)
                    return (qT01, 64 * h) if h < 2 else (t2a, 64)

                def kh(h):
                    return (kT01, 64 * h) if h < 2 else (t2b, 64)

                def unit(kb, h, smalls):
                    # band scores for one (key block, head): two f32 PSUM pieces,
                    # exp -> bf16 SBUF probs, post-exp 0/1 masking on GpSimd.
                    k0 = KB * kb
                    qlo, qhi, llo, lhi = _span(kb)
                    hiA = min(lhi, 512)
                    kt_, ko = kh(h)
                    qt_, qo = qh(h)
                    psA = spsum.tile([128, 512], F32, tag="sc", name=f"sA{kb}_{h}")
                    nc.tensor.matmul(
                        psA[:, llo:hiA],
                        kt_[ko:ko + 64, k0:k0 + KB],
                        qt_[qo:qo + 64, qlo:qlo + (hiA - llo)])
                    for _ in range(2):
                        if smalls:
                            smalls.pop(0)()
                    if llo == 0:
                        nc.vector.tensor_add(psA[:, 0:128], psA[:, 0:128],
                                             masks_t[:, 0:128])
                    t_ = pbtp.tile([128, 640], BF, tag="pb")
                    if lhi > 512:
                        psB = spsum.tile([128, 128], F32, tag="sc", name=f"sB{kb}_{h}")
                        nc.tensor.matmul(
                            psB[:, 0:lhi - 512],
                            kt_[ko:ko + 64, k0:k0 + KB],
                            qt_[qo:qo + 64, qlo + (512 - llo):qlo + (lhi - llo)])
                        nc.vector.tensor_add(psB[:, 0:128], psB[:, 0:128],
                                             masks_t[:, 128:256])
                        nc.scalar.activation(t_[:, 512:lhi], psB[:, 0:lhi - 512], AF.Exp)
                    nc.scalar.activation(t_[:, llo:hiA], psA[:, llo:hiA], AF.Exp)
                    pbt[(kb, h)] = t_

                def do_qb(i):
                    # natural-layout PV for query block i (128 queries, 3 heads + denom)
                    qps = qpsum.tile([128, 195], F32, tag="qp", name=f"qp{i}")
                    kbs = list(range(max(0, i - 2), min(NKB - 1, i + 2) + 1))
                    for h in range(3):
                        for n_, j in enumerate(kbs):
                            la = 128 * (i - j) + 256
                            nc.tensor.matmul(
                                qps[:, 65 * h:65 * h + 65],
                                pbt[(j, h)][:, la:la + 128],
                                v_nat[:, j, h, :],
                                start=(n_ == 0), stop=False)
                        nc.tensor.matmul(
                            qps[:, 65 * h:65 * h + 65],
                            exp_sg[32 * h:32 * h + 32, 128 * i:128 * i + 128],
                            vg3[32 * h:32 * h + 32, :],
                            start=False, stop=True)
                    ob = osbp.tile([128, 195], F32, tag="ob")
                    nc.vector.tensor_copy(ob[:], qps[:])
                    nc.sync.dma_start(out_d[i], ob[:].rearrange("p (h e) -> p h e", h=3))

                do_qb.next = 0

                for nt in range(NNT):
                    c0 = 512 * nt
                    # work to interleave into this round:
                    # - early band units: kb in [4nt-4, 4nt-3] (span needs only cols
                    #   < 512nt, i.e. previous rounds' projections)
                    # - late band units: kb in [4nt-2, 4nt-1] (+28..31 at nt=7), after
                    #   this round's q/k/kg passes are copied
                    early = [(kb, h) for kb in range(max(0, 4 * nt - 4), 4 * nt - 2)
                             for h in range(3) if 0 <= kb < NKB]
                    late = [(kb, h)
                            for kb in range(max(0, 4 * nt - 2),
                                            NKB if nt == NNT - 1 else 4 * nt)
                            for h in range(3)]
                    # small LDW-bound matmuls (probs_g transposes of this round's
                    # chunks + go accumulation over last round's chunks)
                    smalls = []

                    def mk_tr(t):
                        def f():
                            pst = apsum.tile([128, 128], BF, tag="pp", name=f"tr{t}")
                            nc.tensor.transpose(pst[:], probs_g[:, 128 * t:128 * t + 128],
                                                id128_t[:])
                            nc.vector.tensor_copy(pb_gT[:, t, :], pst[:, 0:96])
                        return f

                    gps = None
                    if nt > 0:
                        gps = qpsum.tile([96, 65], F32, tag="qp", name=f"go{nt}")

                        def mk_go(h, t, n_):
                            def f():
                                nc.tensor.matmul(gps[32 * h:32 * h + 32, :],
                                                 pb_gT[:, t, 32 * h:32 * h + 32],
                                                 vg_nat[:, t, h, :],
                                                 start=(n_ == 0), stop=(n_ == 3))
                            return f

                        for h in range(3):
                            for n_, t in enumerate(range(4 * (nt - 1), 4 * nt)):
                                smalls.append(mk_go(h, t, n_))

                    # ---- projections: 5 packed passes, 6 kt each; early band units
                    # and smalls interleaved between the long fills ----
                    dests = [(qT01, 0), (kT01, 1), (kgT01, 2), (t2a, 3), (t2b, 4)]
                    for p, (dst, bc) in enumerate(dests):
                        ps = apsum.tile([128, 512], F32, tag="pp")
                        for kt in range(NKT):
                            nc.tensor.matmul(ps[:], w5_t[:, kt, p, :],
                                             hsT[:, kt, c0:c0 + 512],
                                             start=(kt == 0), stop=(kt == NKT - 1))
                        nc.vector.tensor_scalar_add(
                            dst[:, c0:c0 + 512], ps[:], biasc_t[:, bc:bc + 1])
                        if smalls:
                            smalls.pop(0)()
                        if early:
                            kb, h = early.pop(0)
                            unit(kb, h, smalls)
                    # ---- v / vg (natural layout), bias via broadcast add ----
                    for s4 in range(4):
                        sb = 4 * nt + s4
                        psv = apsum.tile([128, 384], F32, tag="pp")
                        for kt in range(NKT):
                            nc.tensor.matmul(psv[:],
                                             hsT[:, kt, c0 + 128 * s4:c0 + 128 * s4 + 128],
                                             wv_t[:, kt, :],
                                             start=(kt == 0), stop=(kt == NKT - 1))
                        nc.vector.tensor_add(
                            v_nat[:, sb, :, 0:64],
                            psv[:, 0:192].rearrange("p (h e) -> p h e", h=3),
                            biasv_t[:, 0:192].rearrange("p (h e) -> p h e", h=3))
                        nc.vector.tensor_add(
                            vg_nat[:, sb, :, 0:64],
                            psv[:, 192:384].rearrange("p (h e) -> p h e", h=3),
                            biasv_t[:, 192:384].rearrange("p (h e) -> p h e", h=3))
                        if smalls:
                            smalls.pop(0)()
                        if early:
                            kb, h = early.pop(0)
                            unit(kb, h, smalls)
                        elif late and s4 >= 2:
                            kb, h = late.pop(0)
                            unit(kb, h, smalls)
                    if nt == 0:
                        # qg projection (only first G columns of the sequence)
                        psq = apsum.tile([128, G], F32, tag="pp")
                        for kt in range(NKT):
                            nc.tensor.matmul(psq[:], wqg_t[:, kt, 0:128],
                                             hsT[:, kt, 0:G],
                                             start=(kt == 0), stop=(kt == NKT - 1))
                        nc.vector.tensor_scalar_add(qgT01[:], psq[:], biasc_t[:, 5:6])
                        psq2 = apsum.tile([128, G], F32, tag="pp")
                        for kt in range(NKT):
                            nc.tensor.matmul(psq2[0:64, :], wqg_t[:, kt, 128:192],
                                             hsT[:, kt, 0:G],
                                             start=(kt == 0), stop=(kt == NKT - 1))
                        nc.vector.tensor_scalar_add(qgT2b[:], psq2[0:64, :],
                                                    biasc_t[0:64, 6:7])
                        # block-diagonal packed weights for sg and phase-C scores
                        nc.vector.tensor_copy(sgw01[0:64, 0:32], kT01[0:64, 0:G])
                        nc.vector.tensor_copy(sgw01[64:128, 32:64], kT01[64:128, 0:G])
                        nc.vector.tensor_copy(qgw01[0:64, 0:32], qgT01[0:64, 0:G])
                        nc.vector.tensor_copy(qgw01[64:128, 32:64], qgT01[64:128, 0:G])
                        # v of the G global keys stacked at rows 32h (SBUF->SBUF DMA)
                        for h in range(3):
                            nc.gpsimd.dma_start(vg3[32 * h:32 * h + 32, 0:64],
                                                v_nat[0:G, 0, h, 0:64])
                    # ---- sg: all queries vs G global keys (heads packed 0+1 | 2) ----
                    psS = apsum.tile([128, 512], F32, tag="pp")
                    nc.tensor.matmul(psS[:], sgw01[:], qT01[:, c0:c0 + 512])
                    nc.tensor.matmul(psS[64:96, :], t2b[64:128, 0:G],
                                     t2a[64:128, c0:c0 + 512])
                    nc.scalar.activation(exp_sg[:, c0:c0 + 512], psS[0:96, :], AF.Exp)
                    if late:
                        kb, h = late.pop(0)
                        unit(kb, h, smalls)
                    # ---- phase C scores: G global queries vs this chunk of kg ----
                    psC = apsum.tile([128, 512], F32, tag="pp")
                    nc.tensor.matmul(psC[:], qgw01[:], kgT01[:, c0:c0 + 512])
                    nc.tensor.matmul(psC[64:96, :], qgT2b[:], t2a[0:64, c0:c0 + 512])
                    nc.scalar.activation(probs_g[0:96, c0:c0 + 512], psC[0:96, :], AF.Exp)
                    for t in range(4 * nt, 4 * nt + 4):
                        smalls.append(mk_tr(t))
                    # ---- remaining late band units + leftover smalls ----
                    while late:
                        kb, h = late.pop(0)
                        unit(kb, h, smalls)
                    for fn in smalls:
                        fn()
                    if gps is not None:
                        nc.vector.tensor_add(go_acc[:], go_acc[:], gps[:])
                    # ---- PV query blocks; emitted only after the go-add above so
                    # the qp slot rotation can never deadlock on gps ----
                    qb_hi = (NQB - 3) if nt == NNT - 1 else (4 * nt - 3)
                    while do_qb.next <= qb_hi:
                        do_qb(do_qb.next)
                        do_qb.next += 1

                # ---- tail: remaining query blocks + last go chunks ----
                smalls = []
                gps = qpsum.tile([96, 65], F32, tag="qp", name="gotail")

                def mk_go2(h, t, n_):
                    def f():
                        nc.tensor.matmul(gps[32 * h:32 * h + 32, :],
                                         pb_gT[:, t, 32 * h:32 * h + 32],
                                         vg_nat[:, t, h, :],
                                         start=(n_ == 0), stop=(n_ == 3))
                    return f

                for h in range(3):
                    for n_, t in enumerate(range(28, 32)):
                        smalls.append(mk_go2(h, t, n_))
                while do_qb.next < NQB:
                    do_qb(do_qb.next)
                    do_qb.next += 1
                    for _ in range(2):
                        if smalls:
                            smalls.pop(0)()
                for fn in smalls:
                    fn()
                nc.vector.tensor_add(go_acc[:], go_acc[:], gps[:])
                nc.sync.dma_start(outg_d[:], go_acc[:])

    nc.compile()
    return nc


def _prep_inputs(inputs):
    hs = np.asarray(inputs["hidden_states"], dtype=np.float32)
    j = np.arange(KB)
    p = np.arange(KB)[:, None]
    m_lo = np.where(j[None, :] >= p, 0.0, -60.0).astype(np.float32)
    m_hi = np.where(j[None, :] <= p, 0.0, -60.0).astype(np.float32)
    masksNEG = np.concatenate([m_lo, m_hi], axis=1)
    id128 = np.eye(128, dtype=bf16)

    maps = []
    for c in range(8):
        b, hg = c // 4, c % 4
        cols = slice(192 * hg, 192 * hg + 192)
        Wq = np.asarray(inputs["Wq"], np.float32)[:, cols] * SCALE
        bq = np.asarray(inputs["bq"], np.float32)[cols] * SCALE
        Wqg = np.asarray(inputs["Wqg"], np.float32)[:, cols] * SCALE
        bqg = np.asarray(inputs["bqg"], np.float32)[cols] * SCALE
        Wk = np.asarray(inputs["Wk"], np.float32)[:, cols]
        bk = np.asarray(inputs["bk"], np.float32)[cols]
        Wkg = np.asarray(inputs["Wkg"], np.float32)[:, cols]
        bkg = np.asarray(inputs["bkg"], np.float32)[cols]
        Wv = np.asarray(inputs["Wv"], np.float32)[:, cols]
        bv = np.asarray(inputs["bv"], np.float32)[cols]
        Wvg = np.asarray(inputs["Wvg"], np.float32)[:, cols]
        bvg = np.asarray(inputs["bvg"], np.float32)[cols]

        # hidden transposed: [128, kt, s]
        hsT = np.ascontiguousarray(
            hs[b].T.reshape(NKT, 128, S).transpose(1, 0, 2)).astype(bf16)

        # packed q/k/kg weight passes: [128, kt, pass, 128]
        def ktview(Wm):
            return Wm.reshape(NKT, 128, 192)
        Wqk, Wkk, Wgk = ktview(Wq), ktview(Wk), ktview(Wkg)
        w5 = np.zeros((128, NKT, 5, 128), np.float32)
        for kt in range(NKT):
            w5[:, kt, 0, :] = Wqk[kt, :, 0:128]
            w5[:, kt, 1, :] = Wkk[kt, :, 0:128]
            w5[:, kt, 2, :] = Wgk[kt, :, 0:128]
            w5[:, kt, 3, 0:64] = Wgk[kt, :, 128:192]
            w5[:, kt, 3, 64:128] = Wqk[kt, :, 128:192]
            w5[:, kt, 4, 64:128] = Wkk[kt, :, 128:192]
        w5 = w5.astype(bf16)

        wv = np.ascontiguousarray(np.concatenate([Wv, Wvg], axis=1)
                                  .reshape(NKT, 128, 384).transpose(1, 0, 2)).astype(bf16)
        wqg = np.ascontiguousarray(Wqg.reshape(NKT, 128, 192)
                                   .transpose(1, 0, 2)).astype(bf16)

        biasc = np.zeros((128, 8), np.float32)
        biasc[:, 0] = bq[0:128]
        biasc[:, 1] = bk[0:128]
        biasc[:, 2] = bkg[0:128]
        biasc[0:64, 3] = bkg[128:192]
        biasc[64:128, 3] = bq[128:192]
        biasc[64:128, 4] = bk[128:192]
        biasc[:, 5] = bqg[0:128]
        biasc[0:64, 6] = bqg[128:192]

        biasv = np.tile(np.concatenate([bv, bvg])[None, :], (128, 1)).astype(bf16)

        maps.append({
            "hsT": hsT,
            "w5": w5,
            "wv": wv,
            "wqg": wqg,
            "biasc": biasc,
            "biasv": biasv,
            "masksNEG": masksNEG,
            "id128": id128,
        })
    return maps


def kernel(**inputs):
    g = int(np.asarray(inputs["num_global"]))
    assert g == G, f"kernel compiled for num_global=32, got {g}"
    if "nc" not in _cache:
        _cache["nc"] = _build()
    nc = _cache["nc"]
    in_maps = _prep_inputs(inputs)
    res = run_bass_kernel_spmd(nc, in_maps, list(range(8)))
    return assemble(res.results)


def assemble(results):
    out = np.zeros((B, S, D), np.float32)
    for c in range(8):
        b, hg = c // 4, c % 4
        o = results[c]["out"].reshape(S, 3, 65)   # natural layout
        og = results[c]["outg"]                   # [96, 65]
        for h in range(3):
            col = 192 * hg + 64 * h
            out[b, :, col:col + 64] = o[:, h, 0:64] / o[:, h, 64:65]
            out[b, 0:G, col:col + 64] = (og[32 * h:32 * h + 32, 0:64]
                                         / og[32 * h:32 * h + 32, 64:65])
    return out


# revision 17
# speedup vs baseline: 1.1766x; 1.1383x over previous
"""Trainium2 Bass kernel for Longformer self-attention (B=2, S=4096, D=768, H=12, HD=64, W=256, G=32).

Sharding: 8 cores = 2 batches x 4 head-groups (3 heads each). Each core computes its
batch's projections restricted to its 192 output channels, runs banded + global
attention for its 3 heads.

v1 design (vs v0 baseline at ~248us):
 - hidden_states pre-transposed on host -> contiguous DMA loads (no DMA_TRANSPOSE).
 - q/k/kg projections packed into 5 128-col passes/kt instead of 6 (the three 64-col
   remainders share two passes; t2a holds kg-h2 rows 0:64 + q-h2 rows 64:128, t2b
   holds k-h2 rows 64:128).
 - v/vg bias via broadcast tensor_add on the PSUM->SBUF copy (no ones-matmul).
 - band scores in two f32 PSUM pieces (512+128 cols, 1 bank each); window masking
   done POST-exp as 0/1 multiplies on GpSimd (frees Vector+PE).
 - PV in natural layout: probs chunks [128 keys,128 q] are the stationary operand,
   v_nat [128,65] streams (FD=65); output accumulates per 128-query block in PSUM
   [128, 3*65] including the softmax denominator via the ones column.
 - sg (global-key) and phase-C (global-query) score matmuls pack heads 0+1 into one
   block-diagonal 128-row pass; exp_sg/probs_g are [96, S] (rows 32h..32h+32 = head h).
 - phase C (probs transpose + qg x kg scores + go accumulation) runs incrementally;
   band-score units are interleaved between the long projection fills (Scalar exp is
   ~1us/unit vs 0.32us of PE, so units must stay spread), and small LDW-bound matmuls
   (transposes, go) hide their weight loads under long fills.
Host assembles: out[q] = num/den per head; global-query rows replaced from outg.
"""
import numpy as np
import ml_dtypes

import concourse.bass as bass
import concourse.mybir as mybir
import concourse.tile as tile
from concourse import bacc
from concourse.bass_utils import run_bass_kernel_spmd

B, S, D, H, HD = 2, 4096, 768, 12, 64
W = 256
G = 32
SCALE = 1.0 / np.float32(np.sqrt(HD))
KB = 128
NKB = S // KB     # 32 key blocks
NQB = S // KB     # 32 query blocks
NKT = D // 128    # 6
NNT = S // 512    # 8

BF = mybir.dt.bfloat16
F32 = mybir.dt.float32
AF = mybir.ActivationFunctionType
bf16 = ml_dtypes.bfloat16

_cache = {}


def _span(kb):
    # local valid col range [llo, lhi) within the 640-wide band tile of key block kb
    k0 = KB * kb
    qlo, qhi = max(0, k0 - 2 * KB), min(S, k0 + 3 * KB)
    return qlo, qhi, qlo - (k0 - 2 * KB), qhi - (k0 - 2 * KB)


def _build():
    nc = bacc.Bacc(None, target_bir_lowering=False)

    hsT_d = nc.declare_dram_parameter("hsT", [128, NKT, S], BF, isOutput=False)
    w5_d = nc.declare_dram_parameter("w5", [128, NKT, 5, 128], BF, isOutput=False)
    wv_d = nc.declare_dram_parameter("wv", [128, NKT, 384], BF, isOutput=False)
    wqg_d = nc.declare_dram_parameter("wqg", [128, NKT, 192], BF, isOutput=False)
    biasc_d = nc.declare_dram_parameter("biasc", [128, 8], F32, isOutput=False)
    biasv_d = nc.declare_dram_parameter("biasv", [128, 384], BF, isOutput=False)
    masks_d = nc.declare_dram_parameter("masksNEG", [128, 256], F32, isOutput=False)
    id128_d = nc.declare_dram_parameter("id128", [128, 128], BF, isOutput=False)
    out_d = nc.declare_dram_parameter("out", [3, 65, S], F32, isOutput=True)
    outg_d = nc.declare_dram_parameter("outg", [65, 96], F32, isOutput=True)

    with tile.TileContext(nc) as tc:
        with tc.tile_pool(name="persist", bufs=1) as pp:
            # --- persistent SBUF ---
            hsT = pp.tile([128, NKT, S], BF)
            qT01 = pp.tile([128, S], BF)
            kT01 = pp.tile([128, S], BF)
            kgT01 = pp.tile([128, S], BF)
            t2a = pp.tile([128, S], BF)   # rows 0:64 kg-h2, rows 64:128 q-h2
            t2b = pp.tile([128, S], BF)   # rows 64:128 k-h2
            v_nat = pp.tile([128, NKB, 3, 65], BF)
            vg_nat = pp.tile([128, NKB, 3, 65], BF)
            exp_sg = pp.tile([96, S], BF)     # rows 32h.. = head h, exp(q . k_glob)
            probs_g = pp.tile([128, S], BF)   # rows 32h.. = head h, exp(qg . kg); rows 96+ zero
            pb_gT = pp.tile([128, NKB, 96], BF)
            vg3 = pp.tile([96, 65], BF)       # v of global keys, stacked per head
            qgT01 = pp.tile([128, G], BF)
            qgT2b = pp.tile([64, G], BF)
            sgw01 = pp.tile([128, 128], BF)   # block-diag k[:, :G] heads 0|1 (padded)
            qgw01 = pp.tile([128, 128], BF)   # block-diag qg heads 0|1 (padded)
            go_acc = pp.tile([65, 96], F32)

            w5_t = pp.tile([128, NKT, 5, 128], BF)
            wv_t = pp.tile([128, NKT, 384], BF)
            wqg_t = pp.tile([128, NKT, 192], BF)
            biasc_t = pp.tile([128, 8], F32)
            biasv_t = pp.tile([128, 384], BF)
            masks_t = pp.tile([128, 256], F32)
            id128_t = pp.tile([128, 128], BF)

            nc.vector.memset(v_nat[:, :, :, 64:65], 1.0)
            nc.vector.memset(vg_nat[:, :, :, 64:65], 1.0)
            nc.vector.memset(vg3[:, 64:65], 1.0)
            nc.vector.memset(sgw01[:], 0.0)
            nc.vector.memset(qgw01[:], 0.0)
            nc.vector.memset(go_acc[:], 0.0)
            nc.vector.memset(probs_g[96:128, :], 0.0)

            # --- input DMAs on two parallel queues: hidden stream on sync,
            # weights/consts on gpsimd (which later only carries output DMAs) ---
            nc.gpsimd.dma_start(biasc_t[:], biasc_d[:])
            for kt in range(NKT):
                nc.gpsimd.dma_start(w5_t[:, kt], w5_d[:, kt])
            for kt in range(NKT):
                nc.gpsimd.dma_start(wv_t[:, kt], wv_d[:, kt])
            nc.gpsimd.dma_start(biasv_t[:], biasv_d[:])
            nc.gpsimd.dma_start(wqg_t[:], wqg_d[:])
            nc.gpsimd.dma_start(masks_t[:], masks_d[:])
            nc.gpsimd.dma_start(id128_t[:], id128_d[:])
            for nt in range(NNT):
                for kt in range(NKT):
                    c0 = 512 * nt
                    nc.sync.dma_start(hsT[:, kt, c0:c0 + 512], hsT_d[:, kt, c0:c0 + 512])

            with (
                tc.tile_pool(name="apsum", bufs=2, space="PSUM") as apsum,
                tc.tile_pool(name="spsum", bufs=4, space="PSUM") as spsum,
                tc.tile_pool(name="opsum", bufs=2, space="PSUM") as opsum,
                tc.tile_pool(name="pbt", bufs=28) as pbtp,
                tc.tile_pool(name="osb", bufs=4) as osbp,
            ):
                pbt = {}

                def qh(h):  # q of head h: (tile, row offset)
                    return (qT01, 64 * h) if h < 2 else (t2a, 64)

                def kh(h):
                    return (kT01, 64 * h) if h < 2 else (t2b, 64)

                def unit(kb, h, smalls):
                    # band scores for one (key block, head): two f32 PSUM pieces,
                    # exp -> bf16 SBUF probs, post-exp 0/1 masking on GpSimd.
                    k0 = KB * kb
                    qlo, qhi, llo, lhi = _span(kb)
                    hiA = min(lhi, 512)
                    kt_, ko = kh(h)
                    qt_, qo = qh(h)
                    psA = spsum.tile([128, 512], F32, tag="sc", name=f"sA{kb}_{h}")
                    nc.tensor.matmul(
                        psA[:, llo:hiA],
                        kt_[ko:ko + 64, k0:k0 + KB],
                        qt_[qo:qo + 64, qlo:qlo + (hiA - llo)])
                    for _ in range(2):
                        if smalls:
                            smalls.pop(0)()
                    if llo == 0:
                        nc.vector.tensor_add(psA[:, 0:128], psA[:, 0:128],
                                             masks_t[:, 0:128])
                    t_ = pbtp.tile([128, 640], BF, tag="pb")
                    if lhi > 512:
                        psB = spsum.tile([128, 128], F32, tag="sc", name=f"sB{kb}_{h}")
                        nc.tensor.matmul(
                            psB[:, 0:lhi - 512],
                            kt_[ko:ko + 64, k0:k0 + KB],
                            qt_[qo:qo + 64, qlo + (512 - llo):qlo + (lhi - llo)])
                        nc.vector.tensor_add(psB[:, 0:128], psB[:, 0:128],
                                             masks_t[:, 128:256])
                        nc.scalar.activation(t_[:, 512:lhi], psB[:, 0:lhi - 512], AF.Exp)
                    nc.scalar.activation(t_[:, llo:hiA], psA[:, llo:hiA], AF.Exp)
                    pbt[(kb, h)] = t_

                def do_pv(qs):
                    # transposed PV for one 512-query stripe: stationary v (65 cols),
                    # long prob streams; row 64 accumulates the softmax denominator.
                    q0 = 512 * qs
                    kbs = list(range(max(0, 4 * qs - 2), min(NKB, 4 * qs + 6)))
                    for h in range(3):
                        po = opsum.tile([65, 512], F32, tag="po", name=f"po{qs}_{h}")
                        nc.tensor.matmul(po[:], vg3[32 * h:32 * h + 32, :],
                                         exp_sg[32 * h:32 * h + 32, q0:q0 + 512],
                                         start=True, stop=False)
                        for i, kb in enumerate(kbs):
                            k0 = KB * kb
                            qlo, qhi, llo, lhi = _span(kb)
                            a, b2 = max(qlo, q0), min(qhi, q0 + 512)
                            la = a - (k0 - 2 * KB)
                            nc.tensor.matmul(po[:, a - q0:b2 - q0], v_nat[:, kb, h, :],
                                             pbt[(kb, h)][:, la:la + (b2 - a)],
                                             start=False, stop=(i == len(kbs) - 1))
                        ob = osbp.tile([65, 512], F32, tag="ob")
                        nc.vector.tensor_copy(ob[:], po[:])
                        nc.sync.dma_start(out_d[h, :, q0:q0 + 512], ob[:])

                do_pv.next = 0

                for nt in range(NNT):
                    c0 = 512 * nt
                    # work to interleave into this round:
                    # - early band units: kb in [4nt-4, 4nt-3] (span needs only cols
                    #   < 512nt, i.e. previous rounds' projections)
                    # - late band units: kb in [4nt-2, 4nt-1] (+28..31 at nt=7), after
                    #   this round's q/k/kg passes are copied
                    early = [(kb, h) for kb in range(max(0, 4 * nt - 4), 4 * nt - 2)
                             for h in range(3) if 0 <= kb < NKB]
                    late = [(kb, h)
                            for kb in range(max(0, 4 * nt - 2),
                                            NKB if nt == NNT - 1 else 4 * nt)
                            for h in range(3)]
                    # small LDW-bound matmuls (probs_g transposes of this round's
                    # chunks + go accumulation over last round's chunks)
                    smalls = []

                    def mk_tr(t):
                        def f():
                            pst = apsum.tile([128, 128], BF, tag="pp", name=f"tr{t}")
                            nc.tensor.transpose(pst[:], probs_g[:, 128 * t:128 * t + 128],
                                                id128_t[:])
                            nc.vector.tensor_copy(pb_gT[:, t, :], pst[:, 0:96])
                        return f

                    gps = None
                    if nt > 0:
                        gps = opsum.tile([65, 96], F32, tag="po", name=f"go{nt}")

                        def mk_go(h, t, n_):
                            def f():
                                nc.tensor.matmul(gps[:, 32 * h:32 * h + 32],
                                                 vg_nat[:, t, h, :],
                                                 pb_gT[:, t, 32 * h:32 * h + 32],
                                                 start=(n_ == 0), stop=(n_ == 3))
                            return f

                        for h in range(3):
                            for n_, t in enumerate(range(4 * (nt - 1), 4 * nt)):
                                smalls.append(mk_go(h, t, n_))

                    # ---- projections: 5 packed passes, 6 kt each; early band units
                    # and smalls interleaved between the long fills ----
                    dests = [(qT01, 0), (kT01, 1), (kgT01, 2), (t2a, 3), (t2b, 4)]
                    for p, (dst, bc) in enumerate(dests):
                        ps = apsum.tile([128, 512], F32, tag="pp")
                        for kt in range(NKT):
                            nc.tensor.matmul(ps[:], w5_t[:, kt, p, :],
                                             hsT[:, kt, c0:c0 + 512],
                                             start=(kt == 0), stop=(kt == NKT - 1))
                        nc.vector.tensor_scalar_add(
                            dst[:, c0:c0 + 512], ps[:], biasc_t[:, bc:bc + 1])
                        if smalls:
                            smalls.pop(0)()
                        if early:
                            kb, h = early.pop(0)
                            unit(kb, h, smalls)
                    # ---- v / vg (natural layout), bias via broadcast add ----
                    for s4 in range(4):
                        sb = 4 * nt + s4
                        psv = apsum.tile([128, 384], F32, tag="pp")
                        for kt in range(NKT):
                            nc.tensor.matmul(psv[:],
                                             hsT[:, kt, c0 + 128 * s4:c0 + 128 * s4 + 128],
                                             wv_t[:, kt, :],
                                             start=(kt == 0), stop=(kt == NKT - 1))
                        nc.vector.tensor_add(
                            v_nat[:, sb, :, 0:64],
                            psv[:, 0:192].rearrange("p (h e) -> p h e", h=3),
                            biasv_t[:, 0:192].rearrange("p (h e) -> p h e", h=3))
                        nc.vector.tensor_add(
                            vg_nat[:, sb, :, 0:64],
                            psv[:, 192:384].rearrange("p (h e) -> p h e", h=3),
                            biasv_t[:, 192:384].rearrange("p (h e) -> p h e", h=3))
                        if smalls:
                            smalls.pop(0)()
                        if early:
                            kb, h = early.pop(0)
                            unit(kb, h, smalls)
                        elif late and s4 >= 2:
                            kb, h = late.pop(0)
                            unit(kb, h, smalls)
                    if nt == 0:
                        # qg projection (only first G columns of the sequence)
                        psq = apsum.tile([128, G], F32, tag="pp")
                        for kt in range(NKT):
                            nc.tensor.matmul(psq[:], wqg_t[:, kt, 0:128],
                                             hsT[:, kt, 0:G],
                                             start=(kt == 0), stop=(kt == NKT - 1))
                        nc.vector.tensor_scalar_add(qgT01[:], psq[:], biasc_t[:, 5:6])
                        psq2 = apsum.tile([128, G], F32, tag="pp")
                        for kt in range(NKT):
                            nc.tensor.matmul(psq2[0:64, :], wqg_t[:, kt, 128:192],
                                             hsT[:, kt, 0:G],
                                             start=(kt == 0), stop=(kt == NKT - 1))
                        nc.vector.tensor_scalar_add(qgT2b[:], psq2[0:64, :],
                                                    biasc_t[0:64, 6:7])
                        # block-diagonal packed weights for sg and phase-C scores
                        nc.vector.tensor_copy(sgw01[0:64, 0:32], kT01[0:64, 0:G])
                        nc.vector.tensor_copy(sgw01[64:128, 32:64], kT01[64:128, 0:G])
                        nc.vector.tensor_copy(qgw01[0:64, 0:32], qgT01[0:64, 0:G])
                        nc.vector.tensor_copy(qgw01[64:128, 32:64], qgT01[64:128, 0:G])
                        # v of the G global keys stacked at rows 32h (SBUF->SBUF DMA)
                        for h in range(3):
                            nc.gpsimd.dma_start(vg3[32 * h:32 * h + 32, 0:64],
                                                v_nat[0:G, 0, h, 0:64])
                    # ---- sg: all queries vs G global keys (heads packed 0+1 | 2) ----
                    psS = apsum.tile([128, 512], F32, tag="pp")
                    nc.tensor.matmul(psS[:], sgw01[:], qT01[:, c0:c0 + 512])
                    nc.tensor.matmul(psS[64:96, :], t2b[64:128, 0:G],
                                     t2a[64:128, c0:c0 + 512])
                    nc.scalar.activation(exp_sg[:, c0:c0 + 512], psS[0:96, :], AF.Exp)
                    if late:
                        kb, h = late.pop(0)
                        unit(kb, h, smalls)
                    # ---- phase C scores: G global queries vs this chunk of kg ----
                    psC = apsum.tile([128, 512], F32, tag="pp")
                    nc.tensor.matmul(psC[:], qgw01[:], kgT01[:, c0:c0 + 512])
                    nc.tensor.matmul(psC[64:96, :], qgT2b[:], t2a[0:64, c0:c0 + 512])
                    nc.scalar.activation(probs_g[0:96, c0:c0 + 512], psC[0:96, :], AF.Exp)
                    for t in range(4 * nt, 4 * nt + 4):
                        smalls.append(mk_tr(t))
                    # ---- remaining late band units + leftover smalls ----
                    while late:
                        kb, h = late.pop(0)
                        unit(kb, h, smalls)
                    for fn in smalls:
                        fn()
                    if gps is not None:
                        nc.vector.tensor_add(go_acc[:], go_acc[:], gps[:])
                    # ---- PV stripes; emitted only after the go-add above so
                    # the po slot rotation can never deadlock on gps ----
                    qs_hi = (NNT - 1) if nt == NNT - 1 else (nt - 2)
                    while do_pv.next <= qs_hi:
                        do_pv(do_pv.next)
                        do_pv.next += 1

                # ---- tail: last go chunks ----
                gps = opsum.tile([65, 96], F32, tag="po", name="gotail")
                for h in range(3):
                    for n_, t in enumerate(range(28, 32)):
                        nc.tensor.matmul(gps[:, 32 * h:32 * h + 32],
                                         vg_nat[:, t, h, :],
                                         pb_gT[:, t, 32 * h:32 * h + 32],
                                         start=(n_ == 0), stop=(n_ == 3))
                nc.vector.tensor_add(go_acc[:], go_acc[:], gps[:])
                nc.sync.dma_start(outg_d[:], go_acc[:])

    nc.compile()
    return nc


def _prep_inputs(inputs):
    hs = np.asarray(inputs["hidden_states"], dtype=np.float32)
    j = np.arange(KB)
    p = np.arange(KB)[:, None]
    m_lo = np.where(j[None, :] >= p, 0.0, -60.0).astype(np.float32)
    m_hi = np.where(j[None, :] <= p, 0.0, -60.0).astype(np.float32)
    masksNEG = np.concatenate([m_lo, m_hi], axis=1)
    id128 = np.eye(128, dtype=bf16)

    maps = []
    for c in range(8):
        b, hg = c // 4, c % 4
        cols = slice(192 * hg, 192 * hg + 192)
        Wq = np.asarray(inputs["Wq"], np.float32)[:, cols] * SCALE
        bq = np.asarray(inputs["bq"], np.float32)[cols] * SCALE
        Wqg = np.asarray(inputs["Wqg"], np.float32)[:, cols] * SCALE
        bqg = np.asarray(inputs["bqg"], np.float32)[cols] * SCALE
        Wk = np.asarray(inputs["Wk"], np.float32)[:, cols]
        bk = np.asarray(inputs["bk"], np.float32)[cols]
        Wkg = np.asarray(inputs["Wkg"], np.float32)[:, cols]
        bkg = np.asarray(inputs["bkg"], np.float32)[cols]
        Wv = np.asarray(inputs["Wv"], np.float32)[:, cols]
        bv = np.asarray(inputs["bv"], np.float32)[cols]
        Wvg = np.asarray(inputs["Wvg"], np.float32)[:, cols]
        bvg = np.asarray(inputs["bvg"], np.float32)[cols]

        # hidden transposed: [128, kt, s]
        hsT = np.ascontiguousarray(
            hs[b].T.reshape(NKT, 128, S).transpose(1, 0, 2)).astype(bf16)

        # packed q/k/kg weight passes: [128, kt, pass, 128]
        def ktview(Wm):
            return Wm.reshape(NKT, 128, 192)
        Wqk, Wkk, Wgk = ktview(Wq), ktview(Wk), ktview(Wkg)
        w5 = np.zeros((128, NKT, 5, 128), np.float32)
        for kt in range(NKT):
            w5[:, kt, 0, :] = Wqk[kt, :, 0:128]
            w5[:, kt, 1, :] = Wkk[kt, :, 0:128]
            w5[:, kt, 2, :] = Wgk[kt, :, 0:128]
            w5[:, kt, 3, 0:64] = Wgk[kt, :, 128:192]
            w5[:, kt, 3, 64:128] = Wqk[kt, :, 128:192]
            w5[:, kt, 4, 64:128] = Wkk[kt, :, 128:192]
        w5 = w5.astype(bf16)

        wv = np.ascontiguousarray(np.concatenate([Wv, Wvg], axis=1)
                                  .reshape(NKT, 128, 384).transpose(1, 0, 2)).astype(bf16)
        wqg = np.ascontiguousarray(Wqg.reshape(NKT, 128, 192)
                                   .transpose(1, 0, 2)).astype(bf16)

        biasc = np.zeros((128, 8), np.float32)
        biasc[:, 0] = bq[0:128]
        biasc[:, 1] = bk[0:128]
        biasc[:, 2] = bkg[0:128]
        biasc[0:64, 3] = bkg[128:192]
        biasc[64:128, 3] = bq[128:192]
        biasc[64:128, 4] = bk[128:192]
        biasc[:, 5] = bqg[0:128]
        biasc[0:64, 6] = bqg[128:192]

        biasv = np.tile(np.concatenate([bv, bvg])[None, :], (128, 1)).astype(bf16)

        maps.append({
            "hsT": hsT,
            "w5": w5,
            "wv": wv,
            "wqg": wqg,
            "biasc": biasc,
            "biasv": biasv,
            "masksNEG": masksNEG,
            "id128": id128,
        })
    return maps


def kernel(**inputs):
    g = int(np.asarray(inputs["num_global"]))
    assert g == G, f"kernel compiled for num_global=32, got {g}"
    if "nc" not in _cache:
        _cache["nc"] = _build()
    nc = _cache["nc"]
    in_maps = _prep_inputs(inputs)
    res = run_bass_kernel_spmd(nc, in_maps, list(range(8)))
    return assemble(res.results)


def assemble(results):
    out = np.zeros((B, S, D), np.float32)
    for c in range(8):
        b, hg = c // 4, c % 4
        o = results[c]["out"]          # [3, 65, S] transposed
        og = results[c]["outg"]        # [65, 96]
        for h in range(3):
            col = 192 * hg + 64 * h
            out[b, :, col:col + 64] = (o[h, 0:64] / o[h, 64]).T
            out[b, 0:G, col:col + 64] = (og[0:64, 32 * h:32 * h + 32]
                                         / og[64, 32 * h:32 * h + 32]).T
    return out


# revision 19
# speedup vs baseline: 1.2228x; 1.0393x over previous
"""Trainium2 Bass kernel for Longformer self-attention (B=2, S=4096, D=768, H=12, HD=64, W=256, G=32).

Sharding: 8 cores = 2 batches x 4 head-groups (3 heads each). Each core computes its
batch's projections restricted to its 192 output channels, runs banded + global
attention for its 3 heads.

v1 design (vs v0 baseline at ~248us):
 - hidden_states pre-transposed on host -> contiguous DMA loads (no DMA_TRANSPOSE).
 - q/k/kg projections packed into 5 128-col passes/kt instead of 6 (the three 64-col
   remainders share two passes; t2a holds kg-h2 rows 0:64 + q-h2 rows 64:128, t2b
   holds k-h2 rows 64:128).
 - v/vg bias via broadcast tensor_add on the PSUM->SBUF copy (no ones-matmul).
 - band scores in two f32 PSUM pieces (512+128 cols, 1 bank each); window masking
   done POST-exp as 0/1 multiplies on GpSimd (frees Vector+PE).
 - PV in natural layout: probs chunks [128 keys,128 q] are the stationary operand,
   v_nat [128,65] streams (FD=65); output accumulates per 128-query block in PSUM
   [128, 3*65] including the softmax denominator via the ones column.
 - sg (global-key) and phase-C (global-query) score matmuls pack heads 0+1 into one
   block-diagonal 128-row pass; exp_sg/probs_g are [96, S] (rows 32h..32h+32 = head h).
 - phase C (probs transpose + qg x kg scores + go accumulation) runs incrementally;
   band-score units are interleaved between the long projection fills (Scalar exp is
   ~1us/unit vs 0.32us of PE, so units must stay spread), and small LDW-bound matmuls
   (transposes, go) hide their weight loads under long fills.
Host assembles: out[q] = num/den per head; global-query rows replaced from outg.
"""
import numpy as np
import ml_dtypes

import concourse.bass as bass
import concourse.mybir as mybir
import concourse.tile as tile
from concourse import bacc
from concourse.bass_utils import run_bass_kernel_spmd

B, S, D, H, HD = 2, 4096, 768, 12, 64
W = 256
G = 32
SCALE = 1.0 / np.float32(np.sqrt(HD))
KB = 128
NKB = S // KB     # 32 key blocks
NQB = S // KB     # 32 query blocks
NKT = D // 128    # 6
NNT = S // 512    # 8

BF = mybir.dt.bfloat16
F32 = mybir.dt.float32
AF = mybir.ActivationFunctionType
bf16 = ml_dtypes.bfloat16

_cache = {}


def _span(kb):
    # local valid col range [llo, lhi) within the 640-wide band tile of key block kb
    k0 = KB * kb
    qlo, qhi = max(0, k0 - 2 * KB), min(S, k0 + 3 * KB)
    return qlo, qhi, qlo - (k0 - 2 * KB), qhi - (k0 - 2 * KB)


def _build():
    nc = bacc.Bacc(None, target_bir_lowering=False)

    hsT_d = nc.declare_dram_parameter("hsT", [128, NKT, S], BF, isOutput=False)
    w5_d = nc.declare_dram_parameter("w5", [128, NKT, 5, 128], BF, isOutput=False)
    wv_d = nc.declare_dram_parameter("wv", [128, NKT, 384], BF, isOutput=False)
    wqg_d = nc.declare_dram_parameter("wqg", [128, NKT, 192], BF, isOutput=False)
    biasc_d = nc.declare_dram_parameter("biasc", [128, 8], F32, isOutput=False)
    biasv_d = nc.declare_dram_parameter("biasv", [128, 384], BF, isOutput=False)
    masks_d = nc.declare_dram_parameter("masks01", [128, 256], BF, isOutput=False)
    id128_d = nc.declare_dram_parameter("id128", [128, 128], BF, isOutput=False)
    out_d = nc.declare_dram_parameter("out", [3, 65, S], F32, isOutput=True)
    outg_d = nc.declare_dram_parameter("outg", [65, 96], F32, isOutput=True)

    with tile.TileContext(nc) as tc:
        with tc.tile_pool(name="persist", bufs=1) as pp:
            # --- persistent SBUF ---
            hsT = pp.tile([128, NKT, S], BF)
            qT01 = pp.tile([128, S], BF)
            kT01 = pp.tile([128, S], BF)
            kgT01 = pp.tile([128, S], BF)
            t2a = pp.tile([128, S], BF)   # rows 0:64 kg-h2, rows 64:128 q-h2
            t2b = pp.tile([128, S], BF)   # rows 64:128 k-h2
            v_nat = pp.tile([128, NKB, 3, 65], BF)
            vg_nat = pp.tile([128, NKB, 3, 65], BF)
            exp_sg = pp.tile([96, S], BF)     # rows 32h.. = head h, exp(q . k_glob)
            probs_g = pp.tile([128, S], BF)   # rows 32h.. = head h, exp(qg . kg); rows 96+ zero
            pb_gT = pp.tile([128, NKB, 96], BF)
            vg3 = pp.tile([96, 65], BF)       # v of global keys, stacked per head
            qgT01 = pp.tile([128, G], BF)
            qgT2b = pp.tile([64, G], BF)
            sgw01 = pp.tile([128, 128], BF)   # block-diag k[:, :G] heads 0|1 (padded)
            qgw01 = pp.tile([128, 128], BF)   # block-diag qg heads 0|1 (padded)
            go_acc = pp.tile([65, 96], F32)

            w5_t = pp.tile([128, NKT, 5, 128], BF)
            wv_t = pp.tile([128, NKT, 384], BF)
            wqg_t = pp.tile([128, NKT, 192], BF)
            biasc_t = pp.tile([128, 8], F32)
            biasv_t = pp.tile([128, 384], BF)
            masks_t = pp.tile([128, 256], BF)
            id128_t = pp.tile([128, 128], BF)

            nc.vector.memset(v_nat[:, :, :, 64:65], 1.0)
            nc.vector.memset(vg_nat[:, :, :, 64:65], 1.0)
            nc.vector.memset(vg3[:, 64:65], 1.0)
            nc.vector.memset(sgw01[:], 0.0)
            nc.vector.memset(qgw01[:], 0.0)
            nc.vector.memset(go_acc[:], 0.0)
            nc.vector.memset(probs_g[96:128, :], 0.0)

            # --- input DMAs on two parallel queues: hidden stream on sync,
            # weights/consts on gpsimd (which later only carries output DMAs) ---
            nc.gpsimd.dma_start(biasc_t[:], biasc_d[:])
            for kt in range(NKT):
                nc.gpsimd.dma_start(w5_t[:, kt], w5_d[:, kt])
            for kt in range(NKT):
                nc.gpsimd.dma_start(wv_t[:, kt], wv_d[:, kt])
            nc.gpsimd.dma_start(biasv_t[:], biasv_d[:])
            nc.gpsimd.dma_start(wqg_t[:], wqg_d[:])
            nc.gpsimd.dma_start(masks_t[:], masks_d[:])
            nc.gpsimd.dma_start(id128_t[:], id128_d[:])
            for nt in range(NNT):
                for kt in range(NKT):
                    c0 = 512 * nt
                    nc.sync.dma_start(hsT[:, kt, c0:c0 + 512], hsT_d[:, kt, c0:c0 + 512])

            with (
                tc.tile_pool(name="apsum", bufs=2, space="PSUM") as apsum,
                tc.tile_pool(name="spsum", bufs=2, space="PSUM") as spsum,
                tc.tile_pool(name="opsum", bufs=2, space="PSUM") as opsum,
                tc.tile_pool(name="pbt", bufs=28) as pbtp,
                tc.tile_pool(name="osb", bufs=4) as osbp,
            ):
                pbt = {}

                def qh(h):  # q of head h: (tile, row offset)
                    return (qT01, 64 * h) if h < 2 else (t2a, 64)

                def kh(h):
                    return (kT01, 64 * h) if h < 2 else (t2b, 64)

                def unit(kb, h, smalls):
                    # band scores for one (key block, head): one f32 PSUM tile,
                    # one 640-wide matmul, one exp, post-exp 0/1 masks on GpSimd.
                    k0 = KB * kb
                    qlo, qhi, llo, lhi = _span(kb)
                    kt_, ko = kh(h)
                    qt_, qo = qh(h)
                    hiA = min(lhi, 512)
                    ps = spsum.tile([128, 640], F32, tag="sc", name=f"sc{kb}_{h}")
                    nc.tensor.matmul(
                        ps[:, llo:hiA],
                        kt_[ko:ko + 64, k0:k0 + KB],
                        qt_[qo:qo + 64, qlo:qlo + (hiA - llo)])
                    if lhi > 512:
                        nc.tensor.matmul(
                            ps[:, 512:lhi],
                            kt_[ko:ko + 64, k0:k0 + KB],
                            qt_[qo:qo + 64, qlo + (512 - llo):qlo + (lhi - llo)])
                    for _ in range(2):
                        if smalls:
                            smalls.pop(0)()
                    t_ = pbtp.tile([128, 640], BF, tag="pb")
                    nc.scalar.activation(t_[:, llo:lhi], ps[:, llo:lhi], AF.Exp)
                    if llo == 0:
                        nc.gpsimd.tensor_mul(t_[:, 0:128], t_[:, 0:128],
                                             masks_t[:, 0:128])
                    if lhi == 640:
                        nc.gpsimd.tensor_mul(t_[:, 512:640], t_[:, 512:640],
                                             masks_t[:, 128:256])
                    pbt[(kb, h)] = t_

                def do_pv(qs):
                    # transposed PV for one 512-query stripe: stationary v (65 cols),
                    # long prob streams; row 64 accumulates the softmax denominator.
                    q0 = 512 * qs
                    kbs = list(range(max(0, 4 * qs - 2), min(NKB, 4 * qs + 6)))
                    for h in range(3):
                        po = opsum.tile([65, 512], F32, tag="po", name=f"po{qs}_{h}")
                        nc.tensor.matmul(po[:], vg3[32 * h:32 * h + 32, :],
                                         exp_sg[32 * h:32 * h + 32, q0:q0 + 512],
                                         start=True, stop=False)
                        for i, kb in enumerate(kbs):
                            k0 = KB * kb
                            qlo, qhi, llo, lhi = _span(kb)
                            a, b2 = max(qlo, q0), min(qhi, q0 + 512)
                            la = a - (k0 - 2 * KB)
                            nc.tensor.matmul(po[:, a - q0:b2 - q0], v_nat[:, kb, h, :],
                                             pbt[(kb, h)][:, la:la + (b2 - a)],
                                             start=False, stop=(i == len(kbs) - 1))
                        ob = osbp.tile([65, 512], F32, tag="ob")
                        nc.vector.tensor_copy(ob[:], po[:])
                        nc.sync.dma_start(out_d[h, :, q0:q0 + 512], ob[:])

                do_pv.next = 0

                for nt in range(NNT):
                    c0 = 512 * nt
                    # work to interleave into this round:
                    # - early band units: kb in [4nt-4, 4nt-3] (span needs only cols
                    #   < 512nt, i.e. previous rounds' projections)
                    # - late band units: kb in [4nt-2, 4nt-1] (+28..31 at nt=7), after
                    #   this round's q/k/kg passes are copied
                    early = [(kb, h) for kb in range(max(0, 4 * nt - 4), 4 * nt - 2)
                             for h in range(3) if 0 <= kb < NKB]
                    late = [(kb, h)
                            for kb in range(max(0, 4 * nt - 2),
                                            NKB if nt == NNT - 1 else 4 * nt)
                            for h in range(3)]
                    # small LDW-bound matmuls (probs_g transposes of this round's
                    # chunks + go accumulation over last round's chunks)
                    smalls = []

                    def mk_tr(t):
                        def f():
                            pst = apsum.tile([128, 128], BF, tag="pp", name=f"tr{t}")
                            nc.tensor.transpose(pst[:], probs_g[:, 128 * t:128 * t + 128],
                                                id128_t[:])
                            nc.vector.tensor_copy(pb_gT[:, t, :], pst[:, 0:96])
                        return f

                    gps = None
                    if nt > 0:
                        gps = opsum.tile([65, 96], F32, tag="po", name=f"go{nt}")

                        def mk_go(h, t, n_):
                            def f():
                                nc.tensor.matmul(gps[:, 32 * h:32 * h + 32],
                                                 vg_nat[:, t, h, :],
                                                 pb_gT[:, t, 32 * h:32 * h + 32],
                                                 start=(n_ == 0), stop=(n_ == 3))
                            return f

                        for h in range(3):
                            for n_, t in enumerate(range(4 * (nt - 1), 4 * nt)):
                                smalls.append(mk_go(h, t, n_))

                    # ---- projections: 5 packed passes, 6 kt each; early band units
                    # and smalls interleaved between the long fills ----
                    dests = [(qT01, 0), (kT01, 1), (kgT01, 2), (t2a, 3), (t2b, 4)]
                    for p, (dst, bc) in enumerate(dests):
                        ps = apsum.tile([128, 512], F32, tag="pp")
                        for kt in range(NKT):
                            nc.tensor.matmul(ps[:], w5_t[:, kt, p, :],
                                             hsT[:, kt, c0:c0 + 512],
                                             start=(kt == 0), stop=(kt == NKT - 1))
                        nc.vector.tensor_scalar_add(
                            dst[:, c0:c0 + 512], ps[:], biasc_t[:, bc:bc + 1])
                        if smalls:
                            smalls.pop(0)()
                        if early:
                            kb, h = early.pop(0)
                            unit(kb, h, smalls)
                    # ---- v / vg (natural layout), bias via broadcast add ----
                    for s4 in range(4):
                        sb = 4 * nt + s4
                        psv = apsum.tile([128, 384], F32, tag="pp")
                        for kt in range(NKT):
                            nc.tensor.matmul(psv[:],
                                             hsT[:, kt, c0 + 128 * s4:c0 + 128 * s4 + 128],
                                             wv_t[:, kt, :],
                                             start=(kt == 0), stop=(kt == NKT - 1))
                        nc.vector.tensor_add(
                            v_nat[:, sb, :, 0:64],
                            psv[:, 0:192].rearrange("p (h e) -> p h e", h=3),
                            biasv_t[:, 0:192].rearrange("p (h e) -> p h e", h=3))
                        nc.vector.tensor_add(
                            vg_nat[:, sb, :, 0:64],
                            psv[:, 192:384].rearrange("p (h e) -> p h e", h=3),
                            biasv_t[:, 192:384].rearrange("p (h e) -> p h e", h=3))
                        if smalls:
                            smalls.pop(0)()
                        if early:
                            kb, h = early.pop(0)
                            unit(kb, h, smalls)
                        elif late and s4 >= 2:
                            kb, h = late.pop(0)
                            unit(kb, h, smalls)
                    if nt == 0:
                        # qg projection (only first G columns of the sequence)
                        psq = apsum.tile([128, G], F32, tag="pp")
                        for kt in range(NKT):
                            nc.tensor.matmul(psq[:], wqg_t[:, kt, 0:128],
                                             hsT[:, kt, 0:G],
                                             start=(kt == 0), stop=(kt == NKT - 1))
                        nc.vector.tensor_scalar_add(qgT01[:], psq[:], biasc_t[:, 5:6])
                        psq2 = apsum.tile([128, G], F32, tag="pp")
                        for kt in range(NKT):
                            nc.tensor.matmul(psq2[0:64, :], wqg_t[:, kt, 128:192],
                                             hsT[:, kt, 0:G],
                                             start=(kt == 0), stop=(kt == NKT - 1))
                        nc.vector.tensor_scalar_add(qgT2b[:], psq2[0:64, :],
                                                    biasc_t[0:64, 6:7])
                        # block-diagonal packed weights for sg and phase-C scores
                        nc.vector.tensor_copy(sgw01[0:64, 0:32], kT01[0:64, 0:G])
                        nc.vector.tensor_copy(sgw01[64:128, 32:64], kT01[64:128, 0:G])
                        nc.vector.tensor_copy(qgw01[0:64, 0:32], qgT01[0:64, 0:G])
                        nc.vector.tensor_copy(qgw01[64:128, 32:64], qgT01[64:128, 0:G])
                        # v of the G global keys stacked at rows 32h (SBUF->SBUF DMA)
                        for h in range(3):
                            nc.gpsimd.dma_start(vg3[32 * h:32 * h + 32, 0:64],
                                                v_nat[0:G, 0, h, 0:64])
                    # ---- sg: all queries vs G global keys (heads packed 0+1 | 2) ----
                    psS = apsum.tile([128, 512], F32, tag="pp")
                    nc.tensor.matmul(psS[:], sgw01[:], qT01[:, c0:c0 + 512])
                    nc.tensor.matmul(psS[64:96, :], t2b[64:128, 0:G],
                                     t2a[64:128, c0:c0 + 512])
                    nc.scalar.activation(exp_sg[:, c0:c0 + 512], psS[0:96, :], AF.Exp)
                    if late:
                        kb, h = late.pop(0)
                        unit(kb, h, smalls)
                    # ---- phase C scores: G global queries vs this chunk of kg ----
                    psC = apsum.tile([128, 512], F32, tag="pp")
                    nc.tensor.matmul(psC[:], qgw01[:], kgT01[:, c0:c0 + 512])
                    nc.tensor.matmul(psC[64:96, :], qgT2b[:], t2a[0:64, c0:c0 + 512])
                    nc.scalar.activation(probs_g[0:96, c0:c0 + 512], psC[0:96, :], AF.Exp)
                    for t in range(4 * nt, 4 * nt + 4):
                        mk_tr(t)()
                    # ---- remaining late band units + leftover smalls ----
                    while late:
                        kb, h = late.pop(0)
                        unit(kb, h, smalls)
                    for fn in smalls:
                        fn()
                    if gps is not None:
                        nc.vector.tensor_add(go_acc[:], go_acc[:], gps[:])
                    # ---- PV stripes; emitted only after the go-add above so
                    # the po slot rotation can never deadlock on gps ----
                    qs_hi = (NNT - 1) if nt == NNT - 1 else (nt - 2)
                    while do_pv.next <= qs_hi:
                        do_pv(do_pv.next)
                        do_pv.next += 1

                # ---- tail: last go chunks ----
                gps = opsum.tile([65, 96], F32, tag="po", name="gotail")
                for h in range(3):
                    for n_, t in enumerate(range(28, 32)):
                        nc.tensor.matmul(gps[:, 32 * h:32 * h + 32],
                                         vg_nat[:, t, h, :],
                                         pb_gT[:, t, 32 * h:32 * h + 32],
                                         start=(n_ == 0), stop=(n_ == 3))
                nc.vector.tensor_add(go_acc[:], go_acc[:], gps[:])
                nc.sync.dma_start(outg_d[:], go_acc[:])

    nc.compile()
    return nc


def _prep_inputs(inputs):
    hs = np.asarray(inputs["hidden_states"], dtype=np.float32)
    j = np.arange(KB)
    p = np.arange(KB)[:, None]
    m_lo = (j[None, :] >= p).astype(np.float32)
    m_hi = (j[None, :] <= p).astype(np.float32)
    masks01 = np.concatenate([m_lo, m_hi], axis=1).astype(bf16)
    id128 = np.eye(128, dtype=bf16)

    maps = []
    for c in range(8):
        b, hg = c // 4, c % 4
        cols = slice(192 * hg, 192 * hg + 192)
        Wq = np.asarray(inputs["Wq"], np.float32)[:, cols] * SCALE
        bq = np.asarray(inputs["bq"], np.float32)[cols] * SCALE
        Wqg = np.asarray(inputs["Wqg"], np.float32)[:, cols] * SCALE
        bqg = np.asarray(inputs["bqg"], np.float32)[cols] * SCALE
        Wk = np.asarray(inputs["Wk"], np.float32)[:, cols]
        bk = np.asarray(inputs["bk"], np.float32)[cols]
        Wkg = np.asarray(inputs["Wkg"], np.float32)[:, cols]
        bkg = np.asarray(inputs["bkg"], np.float32)[cols]
        Wv = np.asarray(inputs["Wv"], np.float32)[:, cols]
        bv = np.asarray(inputs["bv"], np.float32)[cols]
        Wvg = np.asarray(inputs["Wvg"], np.float32)[:, cols]
        bvg = np.asarray(inputs["bvg"], np.float32)[cols]

        # hidden transposed: [128, kt, s]
        hsT = np.ascontiguousarray(
            hs[b].T.reshape(NKT, 128, S).transpose(1, 0, 2)).astype(bf16)

        # packed q/k/kg weight passes: [128, kt, pass, 128]
        def ktview(Wm):
            return Wm.reshape(NKT, 128, 192)
        Wqk, Wkk, Wgk = ktview(Wq), ktview(Wk), ktview(Wkg)
        w5 = np.zeros((128, NKT, 5, 128), np.float32)
        for kt in range(NKT):
            w5[:, kt, 0, :] = Wqk[kt, :, 0:128]
            w5[:, kt, 1, :] = Wkk[kt, :, 0:128]
            w5[:, kt, 2, :] = Wgk[kt, :, 0:128]
            w5[:, kt, 3, 0:64] = Wgk[kt, :, 128:192]
            w5[:, kt, 3, 64:128] = Wqk[kt, :, 128:192]
            w5[:, kt, 4, 64:128] = Wkk[kt, :, 128:192]
        w5 = w5.astype(bf16)

        wv = np.ascontiguousarray(np.concatenate([Wv, Wvg], axis=1)
                                  .reshape(NKT, 128, 384).transpose(1, 0, 2)).astype(bf16)
        wqg = np.ascontiguousarray(Wqg.reshape(NKT, 128, 192)
                                   .transpose(1, 0, 2)).astype(bf16)

        biasc = np.zeros((128, 8), np.float32)
        biasc[:, 0] = bq[0:128]
        biasc[:, 1] = bk[0:128]
        biasc[:, 2] = bkg[0:128]
        biasc[0:64, 3] = bkg[128:192]
        biasc[64:128, 3] = bq[128:192]
        biasc[64:128, 4] = bk[128:192]
        biasc[:, 5] = bqg[0:128]
        biasc[0:64, 6] = bqg[128:192]

        biasv = np.tile(np.concatenate([bv, bvg])[None, :], (128, 1)).astype(bf16)

        maps.append({
            "hsT": hsT,
            "w5": w5,
            "wv": wv,
            "wqg": wqg,
            "biasc": biasc,
            "biasv": biasv,
            "masks01": masks01,
            "id128": id128,
        })
    return maps


def kernel(**inputs):
    g = int(np.asarray(inputs["num_global"]))
    assert g == G, f"kernel compiled for num_global=32, got {g}"
    if "nc" not in _cache:
        _cache["nc"] = _build()
    nc = _cache["nc"]
    in_maps = _prep_inputs(inputs)
    res = run_bass_kernel_spmd(nc, in_maps, list(range(8)))
    return assemble(res.results)


def assemble(results):
    out = np.zeros((B, S, D), np.float32)
    for c in range(8):
        b, hg = c // 4, c % 4
        o = results[c]["out"]          # [3, 65, S] transposed
        og = results[c]["outg"]        # [65, 96]
        for h in range(3):
            col = 192 * hg + 64 * h
            out[b, :, col:col + 64] = (o[h, 0:64] / o[h, 64]).T
            out[b, 0:G, col:col + 64] = (og[0:64, 32 * h:32 * h + 32]
                                         / og[64, 32 * h:32 * h + 32]).T
    return out


# revision 21
# speedup vs baseline: 1.2643x; 1.0339x over previous
"""Trainium2 Bass kernel for Longformer self-attention (B=2, S=4096, D=768, H=12, HD=64, W=256, G=32).

Sharding: 8 cores = 2 batches x 4 head-groups (3 heads each). Each core computes its
batch's projections restricted to its 192 output channels, runs banded + global
attention for its 3 heads.

v1 design (vs v0 baseline at ~248us):
 - hidden_states pre-transposed on host -> contiguous DMA loads (no DMA_TRANSPOSE).
 - q/k/kg projections packed into 5 128-col passes/kt instead of 6 (the three 64-col
   remainders share two passes; t2a holds kg-h2 rows 0:64 + q-h2 rows 64:128, t2b
   holds k-h2 rows 64:128).
 - v/vg bias via broadcast tensor_add on the PSUM->SBUF copy (no ones-matmul).
 - band scores in two f32 PSUM pieces (512+128 cols, 1 bank each); window masking
   done POST-exp as 0/1 multiplies on GpSimd (frees Vector+PE).
 - PV in natural layout: probs chunks [128 keys,128 q] are the stationary operand,
   v_nat [128,65] streams (FD=65); output accumulates per 128-query block in PSUM
   [128, 3*65] including the softmax denominator via the ones column.
 - sg (global-key) and phase-C (global-query) score matmuls pack heads 0+1 into one
   block-diagonal 128-row pass; exp_sg/probs_g are [96, S] (rows 32h..32h+32 = head h).
 - phase C (probs transpose + qg x kg scores + go accumulation) runs incrementally;
   band-score units are interleaved between the long projection fills (Scalar exp is
   ~1us/unit vs 0.32us of PE, so units must stay spread), and small LDW-bound matmuls
   (transposes, go) hide their weight loads under long fills.
Host assembles: out[q] = num/den per head; global-query rows replaced from outg.
"""
import numpy as np
import ml_dtypes

import concourse.bass as bass
import concourse.mybir as mybir
import concourse.tile as tile
from concourse import bacc
from concourse.bass_utils import run_bass_kernel_spmd

B, S, D, H, HD = 2, 4096, 768, 12, 64
W = 256
G = 32
SCALE = 1.0 / np.float32(np.sqrt(HD))
KB = 128
NKB = S // KB     # 32 key blocks
NQB = S // KB     # 32 query blocks
NKT = D // 128    # 6
NNT = S // 512    # 8

BF = mybir.dt.bfloat16
F32 = mybir.dt.float32
AF = mybir.ActivationFunctionType
bf16 = ml_dtypes.bfloat16

_cache = {}


def _span(kb):
    # local valid col range [llo, lhi) within the 640-wide band tile of key block kb
    k0 = KB * kb
    qlo, qhi = max(0, k0 - 2 * KB), min(S, k0 + 3 * KB)
    return qlo, qhi, qlo - (k0 - 2 * KB), qhi - (k0 - 2 * KB)


def _build():
    nc = bacc.Bacc(None, target_bir_lowering=False)

    hsT_d = nc.declare_dram_parameter("hsT", [128, NKT, S], BF, isOutput=False)
    w5_d = nc.declare_dram_parameter("w5", [128, NKT, 5, 128], BF, isOutput=False)
    wv_d = nc.declare_dram_parameter("wv", [128, NKT, 384], BF, isOutput=False)
    wqg_d = nc.declare_dram_parameter("wqg", [128, NKT, 192], BF, isOutput=False)
    biasc_d = nc.declare_dram_parameter("biasc", [128, 8], F32, isOutput=False)
    biasv_d = nc.declare_dram_parameter("biasv", [128, 384], BF, isOutput=False)
    masks_d = nc.declare_dram_parameter("masks01", [128, 256], BF, isOutput=False)
    id128_d = nc.declare_dram_parameter("id128", [128, 128], BF, isOutput=False)
    out_d = nc.declare_dram_parameter("out", [3, 65, S], BF, isOutput=True)
    outg_d = nc.declare_dram_parameter("outg", [65, 96], F32, isOutput=True)

    with tile.TileContext(nc) as tc:
        with tc.tile_pool(name="persist", bufs=1) as pp:
            # --- persistent SBUF ---
            hsT = pp.tile([128, NKT, S], BF)
            qT01 = pp.tile([128, S], BF)
            kT01 = pp.tile([128, S], BF)
            kgT01 = pp.tile([128, S], BF)
            t2a = pp.tile([128, S], BF)   # rows 0:64 kg-h2, rows 64:128 q-h2
            t2b = pp.tile([128, S], BF)   # rows 64:128 k-h2
            v_nat = pp.tile([128, NKB, 3, 128], BF)   # col 64 = ones, 65+ zero
            vg_nat = pp.tile([128, NKB, 3, 65], BF)
            exp_sg = pp.tile([96, S], BF)     # rows 32h.. = head h, exp(q . k_glob)
            probs_g = pp.tile([128, S], BF)   # rows 32h.. = head h, exp(qg . kg); rows 96+ zero
            pb_gT = pp.tile([128, NKB, 96], BF)
            vg3 = pp.tile([96, 128], BF)      # v of global keys, stacked per head (padded)
            qgT01 = pp.tile([128, G], BF)
            qgT2b = pp.tile([64, G], BF)
            sgw01 = pp.tile([128, 128], BF)   # block-diag k[:, :G] heads 0|1 (padded)
            qgw01 = pp.tile([128, 128], BF)   # block-diag qg heads 0|1 (padded)
            go_acc = pp.tile([65, 96], F32)

            w5_t = pp.tile([128, NKT, 5, 128], BF)
            wv_t = pp.tile([128, NKT, 384], BF)
            wqg_t = pp.tile([128, NKT, 192], BF)
            biasc_t = pp.tile([128, 8], F32)
            biasv_t = pp.tile([128, 384], BF)
            masks_t = pp.tile([128, 256], BF)
            id128_t = pp.tile([128, 128], BF)

            nc.vector.memset(v_nat[:, :, :, 64:128], 0.0)
            nc.vector.memset(v_nat[:, :, :, 64:65], 1.0)
            nc.vector.memset(vg_nat[:, :, :, 64:65], 1.0)
            nc.vector.memset(vg3[:, 64:128], 0.0)
            nc.vector.memset(vg3[:, 64:65], 1.0)
            nc.vector.memset(sgw01[:], 0.0)
            nc.vector.memset(qgw01[:], 0.0)
            nc.vector.memset(go_acc[:], 0.0)
            nc.vector.memset(probs_g[96:128, :], 0.0)

            # --- input DMAs on two parallel queues: hidden stream on sync,
            # weights/consts on gpsimd (which later only carries output DMAs) ---
            nc.gpsimd.dma_start(biasc_t[:], biasc_d[:])
            nc.gpsimd.dma_start(w5_t[:, 0, 0], w5_d[:, 0, 0])
            nc.gpsimd.dma_start(w5_t[:, 0, 1:5], w5_d[:, 0, 1:5])
            for kt in range(1, NKT):
                nc.gpsimd.dma_start(w5_t[:, kt], w5_d[:, kt])
            for kt in range(NKT):
                nc.gpsimd.dma_start(wv_t[:, kt], wv_d[:, kt])
            nc.gpsimd.dma_start(biasv_t[:], biasv_d[:])
            nc.gpsimd.dma_start(wqg_t[:], wqg_d[:])
            nc.gpsimd.dma_start(masks_t[:], masks_d[:])
            nc.gpsimd.dma_start(id128_t[:], id128_d[:])
            for nt in range(NNT):
                for kt in range(NKT):
                    c0 = 512 * nt
                    nc.sync.dma_start(hsT[:, kt, c0:c0 + 512], hsT_d[:, kt, c0:c0 + 512])

            with (
                tc.tile_pool(name="apsum", bufs=2, space="PSUM") as apsum,
                tc.tile_pool(name="spsum", bufs=2, space="PSUM") as spsum,
                tc.tile_pool(name="opsum", bufs=2, space="PSUM") as opsum,
                tc.tile_pool(name="pbt", bufs=26) as pbtp,
                tc.tile_pool(name="osb", bufs=4) as osbp,
            ):
                pbt = {}

                def qh(h):  # q of head h: (tile, row offset)
                    return (qT01, 64 * h) if h < 2 else (t2a, 64)

                def kh(h):
                    return (kT01, 64 * h) if h < 2 else (t2b, 64)

                def unit(kb, h, smalls):
                    # band scores for one (key block, head): one f32 PSUM tile,
                    # one 640-wide matmul, one exp, post-exp 0/1 masks on GpSimd.
                    k0 = KB * kb
                    qlo, qhi, llo, lhi = _span(kb)
                    kt_, ko = kh(h)
                    qt_, qo = qh(h)
                    hiA = min(lhi, 512)
                    ps = spsum.tile([128, 640], F32, tag="sc", name=f"sc{kb}_{h}")
                    nc.tensor.matmul(
                        ps[:, llo:hiA],
                        kt_[ko:ko + 64, k0:k0 + KB],
                        qt_[qo:qo + 64, qlo:qlo + (hiA - llo)])
                    if lhi > 512:
                        nc.tensor.matmul(
                            ps[:, 512:lhi],
                            kt_[ko:ko + 64, k0:k0 + KB],
                            qt_[qo:qo + 64, qlo + (512 - llo):qlo + (lhi - llo)])
                    for _ in range(2):
                        if smalls:
                            smalls.pop(0)()
                    t_ = pbtp.tile([128, 640], BF, tag="pb")
                    nc.scalar.activation(t_[:, llo:lhi], ps[:, llo:lhi], AF.Exp)
                    if llo == 0:
                        nc.gpsimd.tensor_mul(t_[:, 0:128], t_[:, 0:128],
                                             masks_t[:, 0:128])
                    if lhi == 640:
                        nc.gpsimd.tensor_mul(t_[:, 512:640], t_[:, 512:640],
                                             masks_t[:, 128:256])
                    pbt[(kb, h)] = t_
                    if h == 2:
                        unit.kb_done = kb

                def do_pv(qs):
                    # transposed PV for one 512-query stripe: stationary v (65 cols),
                    # long prob streams; row 64 accumulates the softmax denominator.
                    q0 = 512 * qs
                    kbs = list(range(max(0, 4 * qs - 2), min(NKB, 4 * qs + 6)))
                    for h in range(3):
                        po = opsum.tile([128, 512], F32, tag="po", name=f"po{qs}_{h}")
                        nc.tensor.matmul(po[:], vg3[32 * h:32 * h + 32, :],
                                         exp_sg[32 * h:32 * h + 32, q0:q0 + 512],
                                         start=True, stop=False)
                        for i, kb in enumerate(kbs):
                            k0 = KB * kb
                            qlo, qhi, llo, lhi = _span(kb)
                            a, b2 = max(qlo, q0), min(qhi, q0 + 512)
                            la = a - (k0 - 2 * KB)
                            nc.tensor.matmul(po[:, a - q0:b2 - q0], v_nat[:, kb, h, :],
                                             pbt[(kb, h)][:, la:la + (b2 - a)],
                                             start=False, stop=(i == len(kbs) - 1))
                        ob = osbp.tile([65, 512], BF, tag="ob")
                        nc.vector.tensor_copy(ob[:], po[0:65, :])
                        nc.sync.dma_start(out_d[h, :, q0:q0 + 512], ob[:])

                do_pv.next = 0
                unit.kb_done = -1

                for nt in range(NNT):
                    c0 = 512 * nt
                    # work to interleave into this round:
                    # - early band units: kb in [4nt-4, 4nt-3] (span needs only cols
                    #   < 512nt, i.e. previous rounds' projections)
                    # - late band units: kb in [4nt-2, 4nt-1] (+28..31 at nt=7), after
                    #   this round's q/k/kg passes are copied
                    early = [(kb, h) for kb in range(max(0, 4 * nt - 4), 4 * nt - 2)
                             for h in range(3) if 0 <= kb < NKB]
                    late = [(kb, h)
                            for kb in range(max(0, 4 * nt - 2),
                                            NKB if nt == NNT - 1 else 4 * nt)
                            for h in range(3)]
                    # small LDW-bound matmuls (probs_g transposes of this round's
                    # chunks + go accumulation over last round's chunks)
                    smalls = []

                    def mk_tr(t):
                        def f():
                            pst = apsum.tile([128, 128], BF, tag="pp", name=f"tr{t}")
                            nc.tensor.transpose(pst[:], probs_g[:, 128 * t:128 * t + 128],
                                                id128_t[:])
                            nc.vector.tensor_copy(pb_gT[:, t, :], pst[:, 0:96])
                        return f

                    gps = None
                    if nt > 0:
                        gps = opsum.tile([65, 96], F32, tag="po", name=f"go{nt}")

                        def mk_go(h, t, n_):
                            def f():
                                nc.tensor.matmul(gps[:, 32 * h:32 * h + 32],
                                                 vg_nat[:, t, h, :],
                                                 pb_gT[:, t, 32 * h:32 * h + 32],
                                                 start=(n_ == 0), stop=(n_ == 3))
                            return f

                        for h in range(3):
                            for n_, t in enumerate(range(4 * (nt - 1), 4 * nt)):
                                smalls.append(mk_go(h, t, n_))

                    # ---- projections: 5 packed passes, 6 kt each; early band units
                    # and smalls interleaved between the long fills ----
                    dests = [(qT01, 0), (kT01, 1), (kgT01, 2), (t2a, 3), (t2b, 4)]
                    for p, (dst, bc) in enumerate(dests):
                        ps = apsum.tile([128, 512], F32, tag="pp")
                        for kt in range(NKT):
                            nc.tensor.matmul(ps[:], w5_t[:, kt, p, :],
                                             hsT[:, kt, c0:c0 + 512],
                                             start=(kt == 0), stop=(kt == NKT - 1))
                        nc.vector.tensor_scalar_add(
                            dst[:, c0:c0 + 512], ps[:], biasc_t[:, bc:bc + 1])
                        if smalls:
                            smalls.pop(0)()
                        if early:
                            kb, h = early.pop(0)
                            unit(kb, h, smalls)
                    # ---- v / vg (natural layout), bias via broadcast add ----
                    for s4 in range(4):
                        sb = 4 * nt + s4
                        psv = apsum.tile([128, 384], F32, tag="pp")
                        for kt in range(NKT):
                            nc.tensor.matmul(psv[:],
                                             hsT[:, kt, c0 + 128 * s4:c0 + 128 * s4 + 128],
                                             wv_t[:, kt, :],
                                             start=(kt == 0), stop=(kt == NKT - 1))
                        nc.vector.tensor_add(
                            v_nat[:, sb, :, 0:64],
                            psv[:, 0:192].rearrange("p (h e) -> p h e", h=3),
                            biasv_t[:, 0:192].rearrange("p (h e) -> p h e", h=3))
                        nc.vector.tensor_add(
                            vg_nat[:, sb, :, 0:64],
                            psv[:, 192:384].rearrange("p (h e) -> p h e", h=3),
                            biasv_t[:, 192:384].rearrange("p (h e) -> p h e", h=3))
                        if smalls:
                            smalls.pop(0)()
                        if early:
                            kb, h = early.pop(0)
                            unit(kb, h, smalls)
                        elif late and s4 >= 2:
                            kb, h = late.pop(0)
                            unit(kb, h, smalls)
                    if gps is not None:
                        nc.vector.tensor_add(go_acc[:], go_acc[:], gps[:])
                        gps = None

                    def try_pv(qs_hi_now):
                        while do_pv.next <= qs_hi_now and \
                                4 * do_pv.next + 5 <= unit.kb_done:
                            do_pv(do_pv.next)
                            do_pv.next += 1

                    if nt == 0:
                        # qg projection (only first G columns of the sequence)
                        psq = apsum.tile([128, G], F32, tag="pp")
                        for kt in range(NKT):
                            nc.tensor.matmul(psq[:], wqg_t[:, kt, 0:128],
                                             hsT[:, kt, 0:G],
                                             start=(kt == 0), stop=(kt == NKT - 1))
                        nc.vector.tensor_scalar_add(qgT01[:], psq[:], biasc_t[:, 5:6])
                        psq2 = apsum.tile([128, G], F32, tag="pp")
                        for kt in range(NKT):
                            nc.tensor.matmul(psq2[0:64, :], wqg_t[:, kt, 128:192],
                                             hsT[:, kt, 0:G],
                                             start=(kt == 0), stop=(kt == NKT - 1))
                        nc.vector.tensor_scalar_add(qgT2b[:], psq2[0:64, :],
                                                    biasc_t[0:64, 6:7])
                        # block-diagonal packed weights for sg and phase-C scores
                        nc.vector.tensor_copy(sgw01[0:64, 0:32], kT01[0:64, 0:G])
                        nc.vector.tensor_copy(sgw01[64:128, 32:64], kT01[64:128, 0:G])
                        nc.vector.tensor_copy(qgw01[0:64, 0:32], qgT01[0:64, 0:G])
                        nc.vector.tensor_copy(qgw01[64:128, 32:64], qgT01[64:128, 0:G])
                        # v of the G global keys stacked at rows 32h (SBUF->SBUF DMA)
                        for h in range(3):
                            nc.gpsimd.dma_start(vg3[32 * h:32 * h + 32, 0:64],
                                                v_nat[0:G, 0, h, 0:64])
                    # ---- sg: all queries vs G global keys (heads packed 0+1 | 2) ----
                    psS = apsum.tile([128, 512], F32, tag="pp")
                    nc.tensor.matmul(psS[:], sgw01[:], qT01[:, c0:c0 + 512])
                    nc.tensor.matmul(psS[64:96, :], t2b[64:128, 0:G],
                                     t2a[64:128, c0:c0 + 512])
                    nc.scalar.activation(exp_sg[:, c0:c0 + 512], psS[0:96, :], AF.Exp)
                    if late:
                        kb, h = late.pop(0)
                        unit(kb, h, smalls)
                    # ---- phase C scores: G global queries vs this chunk of kg ----
                    psC = apsum.tile([128, 512], F32, tag="pp")
                    nc.tensor.matmul(psC[:], qgw01[:], kgT01[:, c0:c0 + 512])
                    nc.tensor.matmul(psC[64:96, :], qgT2b[:], t2a[0:64, c0:c0 + 512])
                    nc.scalar.activation(probs_g[0:96, c0:c0 + 512], psC[0:96, :], AF.Exp)
                    for t in range(4 * nt, 4 * nt + 4):
                        mk_tr(t)()
                    # ---- remaining late band units, PV stripes as kbs complete ----
                    qs_hi = (NNT - 1) if nt == NNT - 1 else (nt - 2)
                    while late:
                        kb, h = late.pop(0)
                        unit(kb, h, smalls)
                        if h == 2:
                            try_pv(qs_hi)
                    for fn in smalls:
                        fn()
                    try_pv(qs_hi)

                # ---- tail: last go chunks ----
                gps = opsum.tile([65, 96], F32, tag="po", name="gotail")
                for h in range(3):
                    for n_, t in enumerate(range(28, 32)):
                        nc.tensor.matmul(gps[:, 32 * h:32 * h + 32],
                                         vg_nat[:, t, h, :],
                                         pb_gT[:, t, 32 * h:32 * h + 32],
                                         start=(n_ == 0), stop=(n_ == 3))
                nc.vector.tensor_add(go_acc[:], go_acc[:], gps[:])
                nc.sync.dma_start(outg_d[:], go_acc[:])

    nc.compile()
    return nc


def _prep_inputs(inputs):
    hs = np.asarray(inputs["hidden_states"], dtype=np.float32)
    j = np.arange(KB)
    p = np.arange(KB)[:, None]
    m_lo = (j[None, :] >= p).astype(np.float32)
    m_hi = (j[None, :] <= p).astype(np.float32)
    masks01 = np.concatenate([m_lo, m_hi], axis=1).astype(bf16)
    id128 = np.eye(128, dtype=bf16)

    maps = []
    for c in range(8):
        b, hg = c // 4, c % 4
        cols = slice(192 * hg, 192 * hg + 192)
        Wq = np.asarray(inputs["Wq"], np.float32)[:, cols] * SCALE
        bq = np.asarray(inputs["bq"], np.float32)[cols] * SCALE
        Wqg = np.asarray(inputs["Wqg"], np.float32)[:, cols] * SCALE
        bqg = np.asarray(inputs["bqg"], np.float32)[cols] * SCALE
        Wk = np.asarray(inputs["Wk"], np.float32)[:, cols]
        bk = np.asarray(inputs["bk"], np.float32)[cols]
        Wkg = np.asarray(inputs["Wkg"], np.float32)[:, cols]
        bkg = np.asarray(inputs["bkg"], np.float32)[cols]
        Wv = np.asarray(inputs["Wv"], np.float32)[:, cols]
        bv = np.asarray(inputs["bv"], np.float32)[cols]
        Wvg = np.asarray(inputs["Wvg"], np.float32)[:, cols]
        bvg = np.asarray(inputs["bvg"], np.float32)[cols]

        # hidden transposed: [128, kt, s]
        hsT = np.ascontiguousarray(
            hs[b].T.reshape(NKT, 128, S).transpose(1, 0, 2)).astype(bf16)

        # packed q/k/kg weight passes: [128, kt, pass, 128]
        def ktview(Wm):
            return Wm.reshape(NKT, 128, 192)
        Wqk, Wkk, Wgk = ktview(Wq), ktview(Wk), ktview(Wkg)
        w5 = np.zeros((128, NKT, 5, 128), np.float32)
        for kt in range(NKT):
            w5[:, kt, 0, :] = Wqk[kt, :, 0:128]
            w5[:, kt, 1, :] = Wkk[kt, :, 0:128]
            w5[:, kt, 2, :] = Wgk[kt, :, 0:128]
            w5[:, kt, 3, 0:64] = Wgk[kt, :, 128:192]
            w5[:, kt, 3, 64:128] = Wqk[kt, :, 128:192]
            w5[:, kt, 4, 64:128] = Wkk[kt, :, 128:192]
        w5 = w5.astype(bf16)

        wv = np.ascontiguousarray(np.concatenate([Wv, Wvg], axis=1)
                                  .reshape(NKT, 128, 384).transpose(1, 0, 2)).astype(bf16)
        wqg = np.ascontiguousarray(Wqg.reshape(NKT, 128, 192)
                                   .transpose(1, 0, 2)).astype(bf16)

        biasc = np.zeros((128, 8), np.float32)
        biasc[:, 0] = bq[0:128]
        biasc[:, 1] = bk[0:128]
        biasc[:, 2] = bkg[0:128]
        biasc[0:64, 3] = bkg[128:192]
        biasc[64:128, 3] = bq[128:192]
        biasc[64:128, 4] = bk[128:192]
        biasc[:, 5] = bqg[0:128]
        biasc[0:64, 6] = bqg[128:192]

        biasv = np.tile(np.concatenate([bv, bvg])[None, :], (128, 1)).astype(bf16)

        maps.append({
            "hsT": hsT,
            "w5": w5,
            "wv": wv,
            "wqg": wqg,
            "biasc": biasc,
            "biasv": biasv,
            "masks01": masks01,
            "id128": id128,
        })
    return maps


def kernel(**inputs):
    g = int(np.asarray(inputs["num_global"]))
    assert g == G, f"kernel compiled for num_global=32, got {g}"
    if "nc" not in _cache:
        _cache["nc"] = _build()
    nc = _cache["nc"]
    in_maps = _prep_inputs(inputs)
    res = run_bass_kernel_spmd(nc, in_maps, list(range(8)))
    return assemble(res.results)


def assemble(results):
    out = np.zeros((B, S, D), np.float32)
    for c in range(8):
        b, hg = c // 4, c % 4
        o = np.asarray(results[c]["out"], np.float32)   # [3, 65, S] transposed
        og = np.asarray(results[c]["outg"], np.float32) # [65, 96]
        for h in range(3):
            col = 192 * hg + 64 * h
            out[b, :, col:col + 64] = (o[h, 0:64] / o[h, 64]).T
            out[b, 0:G, col:col + 64] = (og[0:64, 32 * h:32 * h + 32]
                                         / og[64, 32 * h:32 * h + 32]).T
    return out
